# revision 1
# baseline (speedup 1.0000x reference)
"""Trainium2 Bass kernel for nn_DSC_86071144612259.

The reference network collapses to a single linear contraction

    u[b, c] = sum_{d<128} sum_{p} W[d, p, c] * y_rev[b, d, p]

where W [128, P, MC] is assembled exactly (float64, on host) from the
small parameter tensors.  The 270 MB y_rev stream is the real work and
is purely DMA bound, so the kernel moves y as *int8* (per-batch-row
scale, absmax/127) -- half the HBM traffic of the fp16 baseline.  The
PE only eats float dtypes (the BIR verifier rejects integer matmuls),
so int8 y is upconverted to fp16 on-chip: the sync HWDGE ring streams
int8 (measured ~375 GB/s with 2 MB descriptors, the per-core HBM
share), and the casts are split between DVE tensor_copy (~1.22
us/chunk) and ACT activation-Copy (~2.0 us/chunk), which together
(~1.32 chunks/us) hide under the stream (~0.7 us/chunk).

The tensor engine chases per chunk with fp16 matmuls accumulating in
fp32 PSUM (4 batch blocks concurrently in disjoint 32-column PE
groups); the per-row dequant scale is applied by the final PSUM->SBUF
tensor_mul, fused with the output copy.  The only numeric loss is the
int8 rounding of y (measured absmax-rel ~9.2e-3 < the 2e-2 gate).

Sharding: pure data parallel over the batch axis across 8 cores (2048
rows each); W and the scale tile are replicated per-core inputs.
"""

import numpy as np

B = 16384      # batch
L = 129        # history length of y_rev
P = 32         # observation dim
MC = 16        # control dim (output)
H = 24         # spectral dim
M = 64         # filter length
NCORES = 8
BS = B // NCORES           # 2048 batch rows per core
KD = 128                   # delays with nonzero weight
K = KD * P                 # 4096 contraction length
NKC = K // 128             # 32 k-chunks of 128 partitions
CW = BS                    # SBUF columns per chunk (2048)
NFREE = 512                # matmul moving free dim (one fp32 PSUM bank)
NB = BS // NFREE           # 4 batch chunks per core

# All 32 chunks arrive int8 on the sync HWDGE ring and are cast to
# fp16 by DVE/ACT.  Groups stay <= 4 chunks (1 MB) so the converters
# never wait long on a batch; singles at head and tail.  GpSimd casts
# are NOT used: ~8 us/chunk AND they drag concurrent DVE casts down to
# the same pace (measured).  GpSimd DMAs (SWDGE Q0) are also out: any
# Q0 traffic collapses the concurrent HWDGE stream (134+161 vs 375
# solo).  A second HWDGE ring (ACT, Q10) is also out: it steals Q1
# bandwidth exactly during the early phase that feeds the converters.
SYNC_GROUPS = [[0], [1, 2], [3, 4], [5, 6, 7], [8, 9, 10, 11],
               [12, 13, 14, 15], [16, 17, 18, 19], [20, 21, 22, 23],
               [24, 25, 26, 27], [28, 29], [30], [31]]

# Measured batched cast rates (ns/chunk): DVE tensor_copy hits a 2x
# mode; ACT activation-Copy runs 1 elem/cycle.
CONV_RATE = {"vector": 1100.0, "scalar": 1830.0}
CONV_ENGINES = ("vector", "scalar")

# Measured Q1 arrival curve (cumulative MB by us) from the HW trace:
# slow queue ramp, then ~420 B/ns steady.
_ARRIVAL = [(10.0, 0.0), (12.0, 0.45), (14.0, 1.2), (16.0, 2.1),
            (18.0, 2.95), (20.0, 3.85), (22.0, 4.75), (24.0, 5.6),
            (26.0, 6.45), (28.0, 7.25), (30.0, 8.05), (31.6, 8.45)]

_CACHE = {}


def _land_time(cum_mb):
    for (t0, b0), (t1, b1) in zip(_ARRIVAL, _ARRIVAL[1:]):
        if cum_mb <= b1:
            return t0 + (t1 - t0) * (cum_mb - b0) / (b1 - b0)
    return _ARRIVAL[-1][0]


def _conv_runs():
    """Greedy DVE/ACT assignment of chunk casts against the measured
    arrival curve, coalescing adjacent same-engine chunks of a group
    into one batched op.  Chunk 31 is forced onto DVE (faster)."""
    grp = {}
    land = {}
    cum = 0.0
    for gi, chunks in enumerate(SYNC_GROUPS):
        cum += len(chunks) * 0.2621
        for ci in chunks:
            grp[ci] = gi
            land[ci] = _land_time(cum) * 1000.0
    free = {e: 10000.0 for e in CONV_ENGINES}
    assign = {}
    for ci in range(NKC):
        if ci == NKC - 1:
            e = "vector"
        else:
            e = min(CONV_ENGINES,
                    key=lambda e: max(free[e], land[ci]) + CONV_RATE[e])
        assign[ci] = e
        free[e] = max(free[e], land[ci]) + CONV_RATE[e]
    runs = []
    for ci in range(NKC):
        if runs and runs[-1][0] == assign[ci] and runs[-1][2] == grp[ci] \
                and runs[-1][1][-1] == ci - 1:
            runs[-1][1].append(ci)
        else:
            runs.append((assign[ci], [ci], grp[ci]))
    return runs


def _build_w(M0, M_tilde, M_0l, M_big, sigma, lambda_e, phi, phi_tilde):
    """Collapse the parameter tensors into W [KD, MC, P] (float64).

    Mirrors reference.py exactly:
      term1: delay 0,      M0
      term2: delays 1..64, sum_i lambda_i^0.25 phi_tilde[j-1,i] M_tilde[i]
      term3: delays 0..63, sum_l sigma_l^0.25  phi[k,l]         M_0l[l]
      term4: delays 1..127 via conv(phi_tilde[:,i], phi[:,l]) and M_big
    """
    f8 = np.float64
    M0 = M0.astype(f8)
    M_tilde = M_tilde.astype(f8)
    M_0l = M_0l.astype(f8)
    M_big = M_big.astype(f8)
    sigma = sigma.astype(f8)
    lambda_e = lambda_e.astype(f8)
    phi = phi.astype(f8)
    phi_tilde = phi_tilde.astype(f8)

    W = np.zeros((KD, MC, P), dtype=f8)
    W[0] += M0
    pt = phi_tilde * (lambda_e ** 0.25)[None, :]
    W[1:M + 1] += np.einsum("ji,icp->jcp", pt, M_tilde)
    ps = phi * (sigma ** 0.25)[None, :]
    W[0:M] += np.einsum("kl,lcp->kcp", ps, M_0l)
    W4 = np.empty((H, H, 2 * M - 1), dtype=f8)
    for i in range(H):
        for l in range(H):
            W4[i, l] = np.convolve(phi_tilde[:, i], phi[:, l])
    scale = (lambda_e[:, None] * sigma[None, :]) ** 0.25
    W[1:2 * M] += np.einsum("ild,ilcp->dcp", W4 * scale[:, :, None], M_big)
    return W


def _get_nc():
    """Build the per-core Bass program (cached)."""
    if "nc" in _CACHE:
        return _CACHE["nc"]
    import concourse.bass as bass
    import concourse.mybir as mybir

    # per-chunk: (engine, run-ordinal on that engine) for matmul waits
    chunk_wait = {}
    runs_of = {e: [] for e in CONV_ENGINES}
    for ename, chunks, gi in _conv_runs():
        runs_of[ename].append((chunks, gi))
        for ci in chunks:
            chunk_wait[ci] = (ename, len(runs_of[ename]))
    assert sorted(chunk_wait) == list(range(NKC))

    nc = bass.Bass("TRN2", target_bir_lowering=False, enable_partition_id=False)
    y8 = nc.dram_tensor("y8", [128, NKC * CW], mybir.dt.int8, kind="ExternalInput")
    w = nc.dram_tensor("w", [128, NKC * MC], mybir.dt.float16, kind="ExternalInput")
    s = nc.dram_tensor("s", [128, NFREE], mybir.dt.float32, kind="ExternalInput")
    ut = nc.dram_tensor("ut", [128, NFREE], mybir.dt.float16, kind="ExternalOutput")

    y8_sb = nc.alloc_sbuf_tensor("y8_sb", [128, NKC * CW], mybir.dt.int8)
    y_sb = nc.alloc_sbuf_tensor("y_sb", [128, NKC * CW], mybir.dt.float16)
    # W pre-swizzled on host: w_sb[p, ki*MC + c] = W_flat[ki*128 + p, c]
    w_sb = nc.alloc_sbuf_tensor("w_sb", [128, NKC * MC], mybir.dt.float16)
    # Dequant tile: s_sb[32*bc + c, j] = s_row[bc*512 + j]
    s_sb = nc.alloc_sbuf_tensor("s_sb", [128, NFREE], mybir.dt.float32)
    # Output striped across partitions: row 32*bc + c holds u^T[c, bc*512+j]
    u_sb = nc.alloc_sbuf_tensor("u_sb", [128, NFREE], mybir.dt.float16)
    # scratch for the ACT activation-table preload dummy
    warm_sb = nc.alloc_sbuf_tensor("warm_sb", [128, 4], mybir.dt.float16)
    ps = nc.alloc_psum_tensor("ps", [128, NFREE], mybir.dt.float32)

    sem_sg = [nc.alloc_semaphore(f"sem_sg{g}") for g in range(len(SYNC_GROUPS))]
    sem_w = nc.alloc_semaphore("sem_w")
    sem_s = nc.alloc_semaphore("sem_s")
    sem_cv = {e: nc.alloc_semaphore(f"sem_cv_{e}") for e in CONV_ENGINES}
    pe_done = nc.alloc_semaphore("pe_done")
    ve_done = nc.alloc_semaphore("ve_done")
    odma = nc.alloc_semaphore("odma")

    def conv_ops(eng, ename):
        lastg = None
        for chunks, gi in runs_of[ename]:
            if gi != lastg:
                eng.wait_ge(sem_sg[gi], 16)
                lastg = gi
            lo, hi = chunks[0] * CW, (chunks[-1] + 1) * CW
            if ename == "scalar":
                op = eng.copy(out=y_sb[:, lo:hi], in_=y8_sb[:, lo:hi])
            else:
                op = eng.tensor_copy(out=y_sb[:, lo:hi], in_=y8_sb[:, lo:hi])
            op.then_inc(sem_cv[ename], 1)

    with nc.Block() as block:

        @block.sync
        def _(sync):
            for g, chunks in enumerate(SYNC_GROUPS):
                lo, hi = chunks[0] * CW, (chunks[-1] + 1) * CW
                sync.dma_start(
                    out=y8_sb[:, lo:hi], in_=y8[:, lo:hi]
                ).then_inc(sem_sg[g], 16)
            sync.wait_ge(ve_done, 1)
            sync.dma_start(
                out=ut[:, :NFREE // 2], in_=u_sb[:, :NFREE // 2]
            ).then_inc(odma, 16)
            sync.wait_ge(odma, 32)

        @block.gpsimd
        def _(gpsimd):
            # the dequant tile is only needed by the final tensor_mul;
            # park its DMA on the otherwise idle gpsimd SWDGE queue
            gpsimd.dma_start(out=s_sb[:, :], in_=s[:, :]).then_inc(sem_s, 16)

        @block.scalar
        def _(scalar):
            # W first (tensor engine blocks on it); then a dummy Copy
            # to pull the ~1.3 us activation-table load out of the
            # first cast's critical path (reads garbage, result unused).
            scalar.dma_start(out=w_sb[:, :], in_=w[:, :]).then_inc(sem_w, 16)
            scalar.copy(out=warm_sb[:, :], in_=y8_sb[:, 0:4])
            conv_ops(scalar, "scalar")
            scalar.wait_ge(ve_done, 2)
            scalar.dma_start(
                out=ut[:, NFREE // 2:], in_=u_sb[:, NFREE // 2:]
            ).then_inc(odma, 16)
            scalar.wait_ge(odma, 32)

        @block.tensor
        def _(tensor):
            tensor.wait_ge(sem_w, 16)

            def wait_chunk(ci):
                e, n = chunk_wait[ci]
                tensor.wait_ge(sem_cv[e], n)

            for ci in range(NKC - 1):
                wait_chunk(ci)
                for bc in range(NB):
                    tensor.matmul(
                        ps[32 * bc:32 * bc + MC, :],
                        w_sb[:, ci * MC:(ci + 1) * MC],
                        y_sb[:, ci * CW + bc * NFREE:ci * CW + (bc + 1) * NFREE],
                        start=(ci == 0),
                        stop=False,
                        tile_position=(0, 32 * bc),
                    )
            # Last chunk in two N=256 halves so the dequant+store of the
            # first half overlaps the second half's matmuls.
            ci = NKC - 1
            wait_chunk(ci)
            for half in range(2):
                lo, hi = half * NFREE // 2, (half + 1) * NFREE // 2
                for bc in range(NB):
                    mm = tensor.matmul(
                        ps[32 * bc:32 * bc + MC, lo:hi],
                        w_sb[:, ci * MC:(ci + 1) * MC],
                        y_sb[:, ci * CW + bc * NFREE + lo:ci * CW + bc * NFREE + hi],
                        start=False,
                        stop=True,
                        tile_position=(0, 32 * bc),
                    )
                    mm.then_inc(pe_done, 1)

        @block.vector
        def _(vector):
            conv_ops(vector, "vector")
            vector.wait_ge(sem_s, 16)
            for half in range(2):
                lo, hi = half * NFREE // 2, (half + 1) * NFREE // 2
                vector.wait_ge(pe_done, NB * (half + 1))
                vector.tensor_mul(
                    out=u_sb[:, lo:hi], in0=ps[:, lo:hi], in1=s_sb[:, lo:hi]
                ).then_inc(ve_done, 1)

    _CACHE["nc"] = nc
    return nc


def _ensure_ntff_hook():
    """bass_utils hard-imports antenv.axon_hooks when BASS_TRACE is set;
    this container's trimmed antenv lacks it.  Register a working stub
    built from trn_agent_boot's ctypes NTFF driver (or a None hook,
    which bass_utils degrades gracefully on)."""
    import importlib.util
    import sys
    import types

    if "antenv.axon_hooks" in sys.modules:
        return
    try:
        if importlib.util.find_spec("antenv.axon_hooks") is not None:
            return
    except (ImportError, ValueError):
        pass
    try:
        from trn_agent_boot.trn_boot import _ntff_profile_via_ctypes

        hook = _ntff_profile_via_ctypes("/opt/axon/libaxon_pjrt.so")
    except Exception:
        hook = None
    mod = types.ModuleType("antenv.axon_hooks")
    mod.get_axon_ntff_profile_hook = lambda: hook
    sys.modules["antenv.axon_hooks"] = mod


def kernel(y_rev, M0, M_tilde, M_0l, M_big, sigma, lambda_e, phi, phi_tilde):
    _ensure_ntff_hook()
    from concourse.bass_utils import run_bass_kernel_spmd

    W = _build_w(M0, M_tilde, M_0l, M_big, sigma, lambda_e, phi, phi_tilde)
    # W_flat[k, c] with k = d*P + p, then swizzled so chunk ki sits at
    # columns [ki*MC, (ki+1)*MC) of a [128, NKC*MC] tile.
    Wf = W.transpose(0, 2, 1).reshape(K, MC)
    Wd = np.ascontiguousarray(
        Wf.reshape(NKC, 128, MC).transpose(1, 0, 2).reshape(128, NKC * MC)
    ).astype(np.float16)

    in_maps = []
    for sh in range(NCORES):
        blk = y_rev[sh * BS:(sh + 1) * BS, :KD, :].reshape(BS, K)  # [b, k]
        srow = (np.abs(blk).max(axis=1) / 127.0).astype(np.float32)  # [BS]
        np.maximum(srow, 1e-30, out=srow)
        q = np.rint(blk / srow[:, None])
        np.clip(q, -127, 127, out=q)
        q = q.astype(np.int8)
        # partition-major DRAM layout: y8[p, ki*CW + j] = q[j, ki*128 + p]
        ytp = np.ascontiguousarray(
            q.T.reshape(NKC, 128, CW).transpose(1, 0, 2).reshape(128, NKC * CW)
        )
        stile = np.empty((128, NFREE), dtype=np.float32)
        for bc in range(NB):
            stile[32 * bc:32 * (bc + 1), :] = srow[None, bc * NFREE:(bc + 1) * NFREE]
        in_maps.append({"y8": ytp, "w": Wd, "s": stile})

    res = run_bass_kernel_spmd(_get_nc(), in_maps, list(range(NCORES)))
    _CACHE["last_result"] = res

    out = np.empty((B, MC), dtype=np.float32)
    for sh in range(NCORES):
        # ut[32*bc + c, j] = u^T[c, bc*512 + j]
        stripes = res.results[sh]["ut"].reshape(NB, 32, NFREE)[:, :MC, :]
        out[sh * BS:(sh + 1) * BS, :] = (
            stripes.transpose(0, 2, 1).reshape(BS, MC).astype(np.float32)
        )
    return out



# revision 5
# speedup vs baseline: 1.0203x; 1.0203x over previous
"""Trainium2 Bass kernel for nn_DSC_86071144612259.

The reference network collapses to a single linear contraction

    u[b, c] = sum_{d<128} sum_{p} W[d, p, c] * y_rev[b, d, p]

where W [128, P, MC] is assembled exactly (float64, on host) from the
small parameter tensors.  The 270 MB y_rev stream is the real work and
is purely DMA bound (~420 B/ns per-core HBM share on the sync HWDGE
ring Q1), so the kernel moves y mostly as *int8* (per-batch-row scale,
absmax/127).  The PE only eats float dtypes, so int8 y is upconverted
to fp16 on-chip by DVE tensor_copy (~1.15 us/chunk) and ACT
activation-Copy (~1.89 us/chunk); their combined rate (~1.40 chunk/us)
is slightly below the stream rate (~1.6 chunk/us), so the LAST 3
k-chunks are sent as fp16 directly (host pre-scales them by 1/s_row so
all chunks share the same per-row normalization): no cast needed, the
PE eats them straight from the DMA, and cast drain ends together with
the stream instead of ~4 us after it.

The per-row dequant scale (and a 2^6 compensation for the W/64 tile,
which keeps PSUM safely inside fp16 range) is applied on the HOST on
the tiny [B, 16] output -- no s tile, no PSUM tensor_mul on device.
The tail is: last fp16 half-chunk lands -> 2 final matmuls -> PSUM ->
SBUF copies (DVE + ACT halves) -> output DMAs (sync + scalar rings).

Sharding: pure data parallel over the batch axis across 8 cores (2048
rows each); W is a replicated per-core input.
"""

import numpy as np

B = 16384      # batch
L = 129        # history length of y_rev
P = 32         # observation dim
MC = 16        # control dim (output)
H = 24         # spectral dim
M = 64         # filter length
NCORES = 8
BS = B // NCORES           # 2048 batch rows per core
KD = 128                   # delays with nonzero weight
K = KD * P                 # 4096 contraction length
NKC = K // 128             # 32 k-chunks of 128 partitions
CW = BS                    # SBUF columns per chunk (2048)
NFREE = 512                # matmul moving free dim (one fp32 PSUM bank)
NB = BS // NFREE           # 4 batch chunks per core

NI8 = 29                   # chunks 0..28 stream as int8 (cast on DVE/ACT)
NFP = NKC - NI8            # chunks 29..31 stream as fp16 (PE-direct)
WSHIFT = 6                 # W tile is W / 2^WSHIFT; host multiplies back

# int8 group structure on the sync HWDGE ring: fine at the head so the
# converters start ASAP, coarse later (casts lag arrivals anyway).
I8_GROUPS = [[0], [1, 2], [3, 4], [5, 6], [7, 8, 9], [10, 11, 12],
             [13, 14, 15, 16], [17, 18, 19, 20], [21, 22, 23, 24],
             [25, 26, 27, 28]]

# Measured batched cast rates (ns/chunk): DVE tensor_copy ~1145,
# ACT activation-Copy ~1890.  GpSimd casts excluded (8 us/chunk AND
# they drag DVE down); SWDGE Q0 / a second HWDGE ring also excluded
# (they collapse Q1 bandwidth) -- all measured in a prior session.
CONV_RATE = {"vector": 1145.0, "scalar": 1890.0}
CONV_FREE = {"vector": 7600.0, "scalar": 9600.0}  # engine-ready times (ns)
CONV_ENGINES = ("vector", "scalar")

# Arrival model from the measured HW trace (trace-base ns): ring Q1
# starts ~6.8 us, ~0.40 MB/us to 10 us, ~0.42 MB/us after.
def _land_time_ns(cum_mb):
    if cum_mb <= 1.25:
        return 6800.0 + cum_mb / 0.00039
    return 10000.0 + (cum_mb - 1.25) / 0.00042


_CACHE = {}


def _conv_runs():
    """Greedy DVE/ACT assignment of the 29 int8 chunk casts against the
    arrival model, coalescing adjacent same-engine chunks of a group
    into one batched op."""
    grp = {}
    land = {}
    cum = 0.0
    for gi, chunks in enumerate(I8_GROUPS):
        cum += len(chunks) * 0.2621
        for ci in chunks:
            grp[ci] = gi
            land[ci] = _land_time_ns(cum) + 350.0  # sem + dispatch latency
    free = dict(CONV_FREE)
    assign = {}
    for ci in range(NI8):
        e = min(CONV_ENGINES,
                key=lambda e: max(free[e], land[ci]) + CONV_RATE[e])
        assign[ci] = e
        free[e] = max(free[e], land[ci]) + CONV_RATE[e]
    runs = []
    for ci in range(NI8):
        if runs and runs[-1][0] == assign[ci] and runs[-1][2] == grp[ci] \
                and runs[-1][1][-1] == ci - 1:
            runs[-1][1].append(ci)
        else:
            runs.append((assign[ci], [ci], grp[ci]))
    return runs


def _build_w(M0, M_tilde, M_0l, M_big, sigma, lambda_e, phi, phi_tilde):
    """Collapse the parameter tensors into W [KD, MC, P] (float64).

    Mirrors reference.py exactly:
      term1: delay 0,      M0
      term2: delays 1..64, sum_i lambda_i^0.25 phi_tilde[j-1,i] M_tilde[i]
      term3: delays 0..63, sum_l sigma_l^0.25  phi[k,l]         M_0l[l]
      term4: delays 1..127 via conv(phi_tilde[:,i], phi[:,l]) and M_big
    """
    f8 = np.float64
    M0 = M0.astype(f8)
    M_tilde = M_tilde.astype(f8)
    M_0l = M_0l.astype(f8)
    M_big = M_big.astype(f8)
    sigma = sigma.astype(f8)
    lambda_e = lambda_e.astype(f8)
    phi = phi.astype(f8)
    phi_tilde = phi_tilde.astype(f8)

    W = np.zeros((KD, MC, P), dtype=f8)
    W[0] += M0
    pt = phi_tilde * (lambda_e ** 0.25)[None, :]
    W[1:M + 1] += np.einsum("ji,icp->jcp", pt, M_tilde)
    ps = phi * (sigma ** 0.25)[None, :]
    W[0:M] += np.einsum("kl,lcp->kcp", ps, M_0l)
    W4 = np.empty((H, H, 2 * M - 1), dtype=f8)
    for i in range(H):
        for l in range(H):
            W4[i, l] = np.convolve(phi_tilde[:, i], phi[:, l])
    scale = (lambda_e[:, None] * sigma[None, :]) ** 0.25
    W[1:2 * M] += np.einsum("ild,ilcp->dcp", W4 * scale[:, :, None], M_big)
    return W


def _get_nc():
    """Build the per-core Bass program (cached)."""
    if "nc" in _CACHE:
        return _CACHE["nc"]
    import concourse.bass as bass
    import concourse.mybir as mybir

    # per-chunk: (engine, run-ordinal on that engine) for matmul waits
    chunk_wait = {}
    runs_of = {e: [] for e in CONV_ENGINES}
    for ename, chunks, gi in _conv_runs():
        runs_of[ename].append((chunks, gi))
        for ci in chunks:
            chunk_wait[ci] = (ename, len(runs_of[ename]))
    assert sorted(chunk_wait) == list(range(NI8))

    nc = bass.Bass("TRN2", target_bir_lowering=False, enable_partition_id=False)
    y8 = nc.dram_tensor("y8", [128, NI8 * CW], mybir.dt.int8, kind="ExternalInput")
    yf = nc.dram_tensor("yf", [128, NFP * CW], mybir.dt.float16, kind="ExternalInput")
    w = nc.dram_tensor("w", [128, NKC * MC], mybir.dt.float16, kind="ExternalInput")
    ut = nc.dram_tensor("ut", [128, NFREE], mybir.dt.float16, kind="ExternalOutput")

    y8_sb = nc.alloc_sbuf_tensor("y8_sb", [128, NI8 * CW], mybir.dt.int8)
    y_sb = nc.alloc_sbuf_tensor("y_sb", [128, NI8 * CW], mybir.dt.float16)
    yf_sb = nc.alloc_sbuf_tensor("yf_sb", [128, NFP * CW], mybir.dt.float16)
    # W pre-swizzled on host: w_sb[p, ki*MC + c] = W_flat[ki*128 + p, c] / 64
    w_sb = nc.alloc_sbuf_tensor("w_sb", [128, NKC * MC], mybir.dt.float16)
    # Output striped across partitions: row 32*bc + c holds u^T[c, bc*512+j]
    u_sb = nc.alloc_sbuf_tensor("u_sb", [128, NFREE], mybir.dt.float16)
    # scratch for the ACT activation-table preload dummy
    warm_sb = nc.alloc_sbuf_tensor("warm_sb", [128, 4], mybir.dt.float16)
    ps = nc.alloc_psum_tensor("ps", [128, NFREE], mybir.dt.float32)

    sem_g = [nc.alloc_semaphore(f"sem_g{g}") for g in range(len(I8_GROUPS))]
    sem_f0 = nc.alloc_semaphore("sem_f0")   # fp16 chunks 29,30
    sem_b0 = nc.alloc_semaphore("sem_b0")   # fp16 chunk 31, bc 0..1
    sem_b1 = nc.alloc_semaphore("sem_b1")   # fp16 chunk 31, bc 2..3
    sem_w = nc.alloc_semaphore("sem_w")
    sem_cv = {e: nc.alloc_semaphore(f"sem_cv_{e}") for e in CONV_ENGINES}
    pe_done = nc.alloc_semaphore("pe_done")
    out_done = nc.alloc_semaphore("out_done")
    odma = nc.alloc_semaphore("odma")

    def conv_ops(eng, ename):
        lastg = None
        for chunks, gi in runs_of[ename]:
            if gi != lastg:
                eng.wait_ge(sem_g[gi], 16)
                lastg = gi
            lo, hi = chunks[0] * CW, (chunks[-1] + 1) * CW
            if ename == "scalar":
                op = eng.copy(out=y_sb[:, lo:hi], in_=y8_sb[:, lo:hi])
            else:
                op = eng.tensor_copy(out=y_sb[:, lo:hi], in_=y8_sb[:, lo:hi])
            op.then_inc(sem_cv[ename], 1)

    with nc.Block() as block:

        @block.sync
        def _(sync):
            for g, chunks in enumerate(I8_GROUPS):
                lo, hi = chunks[0] * CW, (chunks[-1] + 1) * CW
                sync.dma_start(
                    out=y8_sb[:, lo:hi], in_=y8[:, lo:hi]
                ).then_inc(sem_g[g], 16)
            sync.dma_start(
                out=yf_sb[:, 0:2 * CW], in_=yf[:, 0:2 * CW]
            ).then_inc(sem_f0, 16)
            sync.dma_start(
                out=yf_sb[:, 2 * CW:2 * CW + 2 * NFREE],
                in_=yf[:, 2 * CW:2 * CW + 2 * NFREE],
            ).then_inc(sem_b0, 16)
            sync.dma_start(
                out=yf_sb[:, 2 * CW + 2 * NFREE:3 * CW],
                in_=yf[:, 2 * CW + 2 * NFREE:3 * CW],
            ).then_inc(sem_b1, 16)
            sync.wait_ge(out_done, 1)
            sync.dma_start(
                out=ut[:, :NFREE // 2], in_=u_sb[:, :NFREE // 2]
            ).then_inc(odma, 16)
            sync.wait_ge(odma, 32)

        @block.scalar
        def _(scalar):
            # W first (tensor engine blocks on it); then a dummy Copy
            # to pull the ~1.3 us activation-table load out of the
            # first cast's critical path (reads garbage, result unused).
            scalar.dma_start(out=w_sb[:, :], in_=w[:, :]).then_inc(sem_w, 16)
            scalar.copy(out=warm_sb[:, :], in_=y8_sb[:, 0:4])
            conv_ops(scalar, "scalar")
            scalar.wait_ge(out_done, 2)
            scalar.dma_start(
                out=ut[:, NFREE // 2:], in_=u_sb[:, NFREE // 2:]
            ).then_inc(odma, 16)
            scalar.wait_ge(odma, 32)

        @block.tensor
        def _(tensor):
            tensor.wait_ge(sem_w, 16)

            for ci in range(NI8):
                e, n = chunk_wait[ci]
                tensor.wait_ge(sem_cv[e], n)
                for bc in range(NB):
                    tensor.matmul(
                        ps[32 * bc:32 * bc + MC, :],
                        w_sb[:, ci * MC:(ci + 1) * MC],
                        y_sb[:, ci * CW + bc * NFREE:ci * CW + (bc + 1) * NFREE],
                        start=(ci == 0),
                        stop=False,
                        tile_position=(0, 32 * bc),
                    )
            # fp16-direct chunks 29,30 straight from yf_sb
            tensor.wait_ge(sem_f0, 16)
            for ci in (NI8, NI8 + 1):
                fo = (ci - NI8) * CW
                for bc in range(NB):
                    tensor.matmul(
                        ps[32 * bc:32 * bc + MC, :],
                        w_sb[:, ci * MC:(ci + 1) * MC],
                        yf_sb[:, fo + bc * NFREE:fo + (bc + 1) * NFREE],
                        start=False,
                        stop=False,
                        tile_position=(0, 32 * bc),
                    )
            # last chunk (31) lands as two bc-pair sub-DMAs; finish each
            # bc in two N=256 halves so the copies/stores can chase.
            fo = 2 * CW
            ci = NI8 + 2
            for pair, sem_b in ((0, sem_b0), (1, sem_b1)):
                tensor.wait_ge(sem_b, 16)
                for bc in (2 * pair, 2 * pair + 1):
                    for half in range(2):
                        lo = bc * NFREE + half * (NFREE // 2)
                        hi = lo + NFREE // 2
                        plo, phi_ = half * (NFREE // 2), (half + 1) * (NFREE // 2)
                        mm = tensor.matmul(
                            ps[32 * bc:32 * bc + MC, plo:phi_],
                            w_sb[:, ci * MC:(ci + 1) * MC],
                            yf_sb[:, fo + lo:fo + hi],
                            start=False,
                            stop=True,
                            tile_position=(0, 32 * bc),
                        )
                        mm.then_inc(pe_done, 1)

        @block.vector
        def _(vector):
            conv_ops(vector, "vector")
            # half-0 matmuls are pe_done incs 1,3,5,7
            vector.wait_ge(pe_done, 7)
            vector.tensor_copy(
                out=u_sb[:, :NFREE // 2], in_=ps[:, :NFREE // 2]
            ).then_inc(out_done, 1)
            vector.wait_ge(pe_done, 8)
            vector.tensor_copy(
                out=u_sb[:, NFREE // 2:], in_=ps[:, NFREE // 2:]
            ).then_inc(out_done, 1)

    _CACHE["nc"] = nc
    return nc


def _ensure_ntff_hook():
    """bass_utils hard-imports antenv.axon_hooks when BASS_TRACE is set;
    this container's trimmed antenv lacks it.  Register a working stub
    built from trn_agent_boot's ctypes NTFF driver (or a None hook,
    which bass_utils degrades gracefully on)."""
    import importlib.util
    import sys
    import types

    if "antenv.axon_hooks" in sys.modules:
        return
    try:
        if importlib.util.find_spec("antenv.axon_hooks") is not None:
            return
    except (ImportError, ValueError):
        pass
    try:
        from trn_agent_boot.trn_boot import _ntff_profile_via_ctypes

        hook = _ntff_profile_via_ctypes("/opt/axon/libaxon_pjrt.so")
    except Exception:
        hook = None
    mod = types.ModuleType("antenv.axon_hooks")
    mod.get_axon_ntff_profile_hook = lambda: hook
    sys.modules["antenv.axon_hooks"] = mod


def kernel(y_rev, M0, M_tilde, M_0l, M_big, sigma, lambda_e, phi, phi_tilde):
    _ensure_ntff_hook()
    from concourse.bass_utils import run_bass_kernel_spmd

    W = _build_w(M0, M_tilde, M_0l, M_big, sigma, lambda_e, phi, phi_tilde)
    # W_flat[k, c] with k = d*P + p, then swizzled so chunk ki sits at
    # columns [ki*MC, (ki+1)*MC) of a [128, NKC*MC] tile; / 2^WSHIFT
    # keeps PSUM magnitudes well inside fp16 for the output tile.
    Wf = W.transpose(0, 2, 1).reshape(K, MC) * (0.5 ** WSHIFT)
    Wd = np.ascontiguousarray(
        Wf.reshape(NKC, 128, MC).transpose(1, 0, 2).reshape(128, NKC * MC)
    ).astype(np.float16)

    KI = NI8 * 128            # int8-streamed contraction prefix
    in_maps = []
    srows = []
    for sh in range(NCORES):
        blk = y_rev[sh * BS:(sh + 1) * BS, :KD, :].reshape(BS, K)  # [b, k]
        srow = (np.abs(blk).max(axis=1) / 127.0).astype(np.float32)  # [BS]
        np.maximum(srow, 1e-30, out=srow)
        srows.append(srow)
        yn = blk / srow[:, None]                 # |yn| <= 127
        q = np.rint(yn[:, :KI])
        np.clip(q, -127, 127, out=q)
        q = q.astype(np.int8)
        # partition-major DRAM layout: y8[p, ki*CW + j] = q[j, ki*128 + p]
        ytp = np.ascontiguousarray(
            q.T.reshape(NI8, 128, CW).transpose(1, 0, 2).reshape(128, NI8 * CW)
        )
        yftp = np.ascontiguousarray(
            yn[:, KI:].astype(np.float16).T
            .reshape(NFP, 128, CW).transpose(1, 0, 2).reshape(128, NFP * CW)
        )
        in_maps.append({"y8": ytp, "yf": yftp, "w": Wd})

    res = run_bass_kernel_spmd(_get_nc(), in_maps, list(range(NCORES)))
    _CACHE["last_result"] = res

    out = np.empty((B, MC), dtype=np.float32)
    for sh in range(NCORES):
        # ut[32*bc + c, j] = u^T[c, bc*512 + j] / (64 * srow[bc*512+j])
        stripes = res.results[sh]["ut"].reshape(NB, 32, NFREE)[:, :MC, :]
        u = stripes.transpose(0, 2, 1).reshape(BS, MC).astype(np.float32)
        out[sh * BS:(sh + 1) * BS, :] = u * (srows[sh] * float(2 ** WSHIFT))[:, None]
    return out


# revision 7
# speedup vs baseline: 1.1455x; 1.1228x over previous
"""Trainium2 Bass kernel for nn_DSC_86071144612259.

The reference network collapses to a single linear contraction

    u[b, c] = sum_{d<128} sum_{p} W[d, p, c] * y_rev[b, d, p]

where W [128, P, MC] is assembled exactly (float64, on host) from the
small parameter tensors.  The 270 MB y_rev stream is the real work and
is purely HBM bound (~0.45 MB/us per-core share on the sync HWDGE ring
Q1), so y moves as 1 byte/element.  The PE only eats float dtypes, so
bytes are widened to fp16 on-chip.  Measured facts driving the design
(all from HW traces on this problem):

* DMA-completion semaphores become visible to waiting engines ~2.9 us
  after the queue finishes (in-flight pipeline), so every DMA-gated
  stage pays that once; engine-to-engine semaphores are fast (~0.2 us).
* Direct DVE tensor_copy int8->fp16 runs 1.14 us/chunk ([128, 2048]),
  ACT activation-Copy 1.89 us/chunk; together (1.4 chunk/us) they are
  SLOWER than the arrival rate (1.7 chunk/us) and become the critical
  chain.  Instead, DVE chunks stream as *biased uint8* (y+128) and are
  widened by two dual-op tensor_scalars on uint16 views:
      lo = (v & 0x00FF) | 0x6400 ;  hi = (v >> 8) | 0x6400
  0x6400 | b is exactly fp16(1024 + b), so the fp16 lane holds
  y + 128 + 1024 exactly -- and the constant 1152*sum(W) per output
  channel is removed by the final PSUM->SBUF copy, fused as a
  per-partition tensor_scalar subtract.  The packed ops hit the DVE
  2x/4x mode: 0.67 us/chunk, 1.7x the direct cast.
* The last k-chunk is sent as fp16 directly (host pre-scales by
  1/s_row) so the tail pays DMA-visibility + matmul only, no cast.

The per-row dequant scale (and 2^6 for the W/64 tile, which keeps the
biased PSUM inside fp32 headroom) is applied on the HOST on the tiny
[B, 16] output.  Sharding: pure data parallel over batch across 8
cores (2048 rows each); W is a replicated per-core input.
"""

import numpy as np

B = 16384      # batch
L = 129        # history length of y_rev
P = 32         # observation dim
MC = 16        # control dim (output)
H = 24         # spectral dim
M = 64         # filter length
NCORES = 8
BS = B // NCORES           # 2048 batch rows per core
KD = 128                   # delays with nonzero weight
K = KD * P                 # 4096 contraction length
NKC = K // 128             # 32 k-chunks of 128 partitions
CW = BS                    # SBUF columns per chunk (2048)
NFREE = 512                # matmul moving free dim (one fp32 PSUM bank)
NB = BS // NFREE           # 4 batch chunks per core

NI8 = 31                   # chunks 0..30 stream as bytes (DVE/ACT widen)
WSHIFT = 6                 # W tile is W / 2^WSHIFT; host multiplies back

# byte-chunk group structure on the sync HWDGE ring: fine at the head
# (widening starts ASAP after the ~2.9 us visibility lag), coarse in
# the middle, fine again at the tail (drain granularity).
I8_GROUPS = [[0], [1, 2], [3, 4], [5, 6, 7], [8, 9, 10, 11],
             [12, 13, 14, 15], [16, 17, 18, 19], [20, 21, 22, 23],
             [24, 25, 26], [27, 28], [29], [30]]

# Measured widen rates (ns/chunk): DVE packed dual-op pair ~700,
# ACT activation-Copy ~1890.  GpSimd excluded (8 us/chunk AND it drags
# DVE down); SWDGE Q0 / second HWDGE ring excluded (collapse Q1).
CONV_RATE = {"vector": 700.0, "scalar": 1890.0}
CONV_FREE = {"vector": 7600.0, "scalar": 9600.0}  # engine-ready (ns)
CONV_ENGINES = ("vector", "scalar")

VIS_NS = 2900.0            # DMA sem -> engine visibility lag
SEM0_NS = 8490.0           # first group's sem time (measured)
RATE_MB_NS = 0.4465e-3     # stream pace MB/ns


def _land_time_ns(cum_mb):
    return SEM0_NS + (cum_mb - 0.262) / RATE_MB_NS + VIS_NS


_CACHE = {}


def _conv_runs():
    """Greedy DVE/ACT assignment of the byte-chunk widens against the
    measured arrival model."""
    grp = {}
    land = {}
    cum = 0.0
    for gi, chunks in enumerate(I8_GROUPS):
        cum += len(chunks) * 0.2621
        for ci in chunks:
            grp[ci] = gi
            land[ci] = _land_time_ns(cum)
    free = dict(CONV_FREE)
    assign = {}
    for ci in range(NI8):
        e = min(CONV_ENGINES,
                key=lambda e: max(free[e], land[ci]) + CONV_RATE[e])
        assign[ci] = e
        free[e] = max(free[e], land[ci]) + CONV_RATE[e]
    runs = [(assign[ci], ci, grp[ci]) for ci in range(NI8)]
    return runs


def _dve_chunks():
    return sorted(ci for e, ci, g in _conv_runs() if e == "vector")


def _build_w(M0, M_tilde, M_0l, M_big, sigma, lambda_e, phi, phi_tilde):
    """Collapse the parameter tensors into W [KD, MC, P] (float64).

    Mirrors reference.py exactly:
      term1: delay 0,      M0
      term2: delays 1..64, sum_i lambda_i^0.25 phi_tilde[j-1,i] M_tilde[i]
      term3: delays 0..63, sum_l sigma_l^0.25  phi[k,l]         M_0l[l]
      term4: delays 1..127 via conv(phi_tilde[:,i], phi[:,l]) and M_big
    """
    f8 = np.float64
    M0 = M0.astype(f8)
    M_tilde = M_tilde.astype(f8)
    M_0l = M_0l.astype(f8)
    M_big = M_big.astype(f8)
    sigma = sigma.astype(f8)
    lambda_e = lambda_e.astype(f8)
    phi = phi.astype(f8)
    phi_tilde = phi_tilde.astype(f8)

    W = np.zeros((KD, MC, P), dtype=f8)
    W[0] += M0
    pt = phi_tilde * (lambda_e ** 0.25)[None, :]
    W[1:M + 1] += np.einsum("ji,icp->jcp", pt, M_tilde)
    ps = phi * (sigma ** 0.25)[None, :]
    W[0:M] += np.einsum("kl,lcp->kcp", ps, M_0l)
    W4 = np.empty((H, H, 2 * M - 1), dtype=f8)
    for i in range(H):
        for l in range(H):
            W4[i, l] = np.convolve(phi_tilde[:, i], phi[:, l])
    scale = (lambda_e[:, None] * sigma[None, :]) ** 0.25
    W[1:2 * M] += np.einsum("ild,ilcp->dcp", W4 * scale[:, :, None], M_big)
    return W


def _get_nc():
    """Build the per-core Bass program (cached)."""
    if "nc" in _CACHE:
        return _CACHE["nc"]
    import concourse.bass as bass
    import concourse.mybir as mybir

    runs = _conv_runs()
    # per-chunk: (engine, run-ordinal on that engine) for matmul waits
    chunk_wait = {}
    count = {e: 0 for e in CONV_ENGINES}
    for ename, ci, gi in runs:
        count[ename] += 1
        chunk_wait[ci] = (ename, count[ename])
    assert sorted(chunk_wait) == list(range(NI8))

    nc = bass.Bass("TRN2", target_bir_lowering=False, enable_partition_id=False)
    y8 = nc.dram_tensor("y8", [128, NI8 * CW], mybir.dt.int8, kind="ExternalInput")
    yf = nc.dram_tensor("yf", [128, CW], mybir.dt.float16, kind="ExternalInput")
    # w columns 0..511: swizzled W/64 fp16; columns 512..513: the fp32
    # bias-correction vector (1152*sum_dve(W)) bit-packed as 2 fp16 cols.
    w = nc.dram_tensor("w", [128, NKC * MC + 2], mybir.dt.float16,
                       kind="ExternalInput")
    ut = nc.dram_tensor("ut", [128, NFREE], mybir.dt.float16, kind="ExternalOutput")

    y8_sb = nc.alloc_sbuf_tensor("y8_sb", [128, NI8 * CW], mybir.dt.int8)
    y_sb = nc.alloc_sbuf_tensor("y_sb", [128, NI8 * CW], mybir.dt.float16)
    yf_sb = nc.alloc_sbuf_tensor("yf_sb", [128, CW], mybir.dt.float16)
    w_sb = nc.alloc_sbuf_tensor("w_sb", [128, NKC * MC + 2], mybir.dt.float16)
    u_sb = nc.alloc_sbuf_tensor("u_sb", [128, NFREE], mybir.dt.float16)
    warm_sb = nc.alloc_sbuf_tensor("warm_sb", [128, 4], mybir.dt.float16)
    ps = nc.alloc_psum_tensor("ps", [128, NFREE], mybir.dt.float32)

    sem_g = [nc.alloc_semaphore(f"sem_g{g}") for g in range(len(I8_GROUPS))]
    sem_f = nc.alloc_semaphore("sem_f")     # fp16 chunk 31
    sem_w = nc.alloc_semaphore("sem_w")
    sem_cv = {e: nc.alloc_semaphore(f"sem_cv_{e}") for e in CONV_ENGINES}
    pe_done = nc.alloc_semaphore("pe_done")
    out_done = nc.alloc_semaphore("out_done")
    odma = nc.alloc_semaphore("odma")

    corr_ap = w_sb[:, NKC * MC:NKC * MC + 2].bitcast(mybir.dt.float32)

    def conv_ops(eng, ename):
        lastg = None
        for ename_r, ci, gi in runs:
            if ename_r != ename:
                continue
            if gi != lastg:
                eng.wait_ge(sem_g[gi], 16)
                lastg = gi
            lo, hi = ci * CW, (ci + 1) * CW
            if ename == "scalar":
                eng.copy(
                    out=y_sb[:, lo:hi], in_=y8_sb[:, lo:hi]
                ).then_inc(sem_cv[ename], 1)
            else:
                xv = y8_sb[:, lo:hi].bitcast(mybir.dt.uint16)
                lov = y_sb[:, lo:lo + CW // 2].bitcast(mybir.dt.uint16)
                hiv = y_sb[:, lo + CW // 2:hi].bitcast(mybir.dt.uint16)
                eng.tensor_scalar(
                    out=lov, in0=xv, scalar1=0x00FF, scalar2=0x6400,
                    op0=mybir.AluOpType.bitwise_and,
                    op1=mybir.AluOpType.bitwise_or,
                )
                eng.tensor_scalar(
                    out=hiv, in0=xv, scalar1=8, scalar2=0x6400,
                    op0=mybir.AluOpType.logical_shift_right,
                    op1=mybir.AluOpType.bitwise_or,
                ).then_inc(sem_cv[ename], 1)

    with nc.Block() as block:

        @block.sync
        def _(sync):
            for g, chunks in enumerate(I8_GROUPS):
                lo, hi = chunks[0] * CW, (chunks[-1] + 1) * CW
                sync.dma_start(
                    out=y8_sb[:, lo:hi], in_=y8[:, lo:hi]
                ).then_inc(sem_g[g], 16)
            sync.dma_start(out=yf_sb[:, :], in_=yf[:, :]).then_inc(sem_f, 16)
            sync.wait_ge(out_done, 1)
            sync.dma_start(
                out=ut[:, :NFREE // 2], in_=u_sb[:, :NFREE // 2]
            ).then_inc(odma, 16)

        @block.scalar
        def _(scalar):
            # W first (tensor engine blocks on it); then a dummy Copy
            # to pull the ~1.3 us activation-table load out of the
            # first cast's critical path (reads garbage, result unused).
            scalar.dma_start(out=w_sb[:, :], in_=w[:, :]).then_inc(sem_w, 16)
            scalar.copy(out=warm_sb[:, :], in_=y8_sb[:, 0:4])
            conv_ops(scalar, "scalar")
            scalar.wait_ge(out_done, 2)
            scalar.dma_start(
                out=ut[:, NFREE // 2:], in_=u_sb[:, NFREE // 2:]
            ).then_inc(odma, 16)

        @block.tensor
        def _(tensor):
            tensor.wait_ge(sem_w, 16)

            for ci in range(NI8):
                e, n = chunk_wait[ci]
                tensor.wait_ge(sem_cv[e], n)
                for bc in range(NB):
                    tensor.matmul(
                        ps[32 * bc:32 * bc + MC, :],
                        w_sb[:, ci * MC:(ci + 1) * MC],
                        y_sb[:, ci * CW + bc * NFREE:ci * CW + (bc + 1) * NFREE],
                        start=(ci == 0),
                        stop=False,
                        tile_position=(0, 32 * bc),
                    )
            # last chunk fp16-direct; finish each bc in two N=256 halves
            # so the fused subtract-copies and stores can chase.
            ci = NI8
            tensor.wait_ge(sem_f, 16)
            for bc in range(NB):
                for half in range(2):
                    lo = bc * NFREE + half * (NFREE // 2)
                    hi = lo + NFREE // 2
                    plo, phi_ = half * (NFREE // 2), (half + 1) * (NFREE // 2)
                    tensor.matmul(
                        ps[32 * bc:32 * bc + MC, plo:phi_],
                        w_sb[:, ci * MC:(ci + 1) * MC],
                        yf_sb[:, lo:hi],
                        start=False,
                        stop=True,
                        tile_position=(0, 32 * bc),
                    ).then_inc(pe_done, 1)

        @block.vector
        def _(vector):
            conv_ops(vector, "vector")
            # half-0 matmuls are pe_done incs 1,3,5,7
            import concourse.mybir as mybir
            vector.wait_ge(pe_done, 7)
            vector.tensor_scalar(
                out=u_sb[:, :NFREE // 2], in0=ps[:, :NFREE // 2],
                scalar1=corr_ap, scalar2=None,
                op0=mybir.AluOpType.subtract,
            ).then_inc(out_done, 1)
            vector.wait_ge(pe_done, 8)
            vector.tensor_scalar(
                out=u_sb[:, NFREE // 2:], in0=ps[:, NFREE // 2:],
                scalar1=corr_ap, scalar2=None,
                op0=mybir.AluOpType.subtract,
            ).then_inc(out_done, 1)

    _CACHE["nc"] = nc
    return nc


def _ensure_ntff_hook():
    """bass_utils hard-imports antenv.axon_hooks when BASS_TRACE is set;
    this container's trimmed antenv lacks it.  Register a working stub
    built from trn_agent_boot's ctypes NTFF driver (or a None hook,
    which bass_utils degrades gracefully on)."""
    import importlib.util
    import sys
    import types

    if "antenv.axon_hooks" in sys.modules:
        return
    try:
        if importlib.util.find_spec("antenv.axon_hooks") is not None:
            return
    except (ImportError, ValueError):
        pass
    try:
        from trn_agent_boot.trn_boot import _ntff_profile_via_ctypes

        hook = _ntff_profile_via_ctypes("/opt/axon/libaxon_pjrt.so")
    except Exception:
        hook = None
    mod = types.ModuleType("antenv.axon_hooks")
    mod.get_axon_ntff_profile_hook = lambda: hook
    sys.modules["antenv.axon_hooks"] = mod


def kernel(y_rev, M0, M_tilde, M_0l, M_big, sigma, lambda_e, phi, phi_tilde):
    _ensure_ntff_hook()
    from concourse.bass_utils import run_bass_kernel_spmd

    W = _build_w(M0, M_tilde, M_0l, M_big, sigma, lambda_e, phi, phi_tilde)
    # W_flat[k, c] with k = d*P + p, then swizzled so chunk ki sits at
    # columns [ki*MC, (ki+1)*MC) of a [128, NKC*MC] tile; / 2^WSHIFT
    # keeps PSUM magnitudes inside fp16 range for the output tile.
    Wf = W.transpose(0, 2, 1).reshape(K, MC) * (0.5 ** WSHIFT)
    Wsw = Wf.reshape(NKC, 128, MC).transpose(1, 0, 2).reshape(128, NKC * MC)
    Wd = np.ascontiguousarray(Wsw).astype(np.float16)

    # bias correction: DVE chunks carry y+128, widened to 1024+(y+128) =
    # y + 1152, so PSUM holds u' + 1152*sum_{k in DVE chunks} Wd[k, c].
    dve = _dve_chunks()
    Wd64 = Wd.astype(np.float64)           # the values actually multiplied
    corr = np.zeros(128, dtype=np.float64)  # [32*bc + c] layout, same per bc
    csum = np.zeros(MC, dtype=np.float64)
    for ci in dve:
        csum += 1152.0 * Wd64.reshape(128, NKC, MC)[:, ci, :].sum(axis=0)
    for bc in range(NB):
        corr[32 * bc:32 * bc + MC] = csum
    w_ext = np.empty((128, NKC * MC + 2), dtype=np.float16)
    w_ext[:, :NKC * MC] = Wd
    w_ext[:, NKC * MC:] = (
        corr.astype(np.float32).view(np.float16).reshape(128, 2)
    )

    dve_set = set(dve)
    KI = NI8 * 128            # byte-streamed contraction prefix
    in_maps = []
    srows = []
    for sh in range(NCORES):
        blk = y_rev[sh * BS:(sh + 1) * BS, :KD, :].reshape(BS, K)  # [b, k]
        srow = (np.abs(blk).max(axis=1) / 127.0).astype(np.float32)  # [BS]
        np.maximum(srow, 1e-30, out=srow)
        srows.append(srow)
        yn = blk / srow[:, None]                 # |yn| <= 127
        q = np.rint(yn[:, :KI])
        np.clip(q, -127, 127, out=q)
        q = q.astype(np.int8)
        # partition-major layout per chunk: tile[p, j] = q[j, ki*128 + p];
        # DVE chunks additionally biased +128 and column-interleaved so the
        # packed unpack writes halves [0:1024]=even input cols, [1024:2048].
        ytp = np.empty((128, NI8 * CW), dtype=np.int8)
        qT = q.T.reshape(NI8, 128, CW)           # [ki, p, j]
        for ci in range(NI8):
            t = qT[ci]
            if ci in dve_set:
                bt = (t.astype(np.int16) + 128).astype(np.uint8)
                it = np.empty((128, CW), dtype=np.uint8)
                it[:, 0::2] = bt[:, :CW // 2]
                it[:, 1::2] = bt[:, CW // 2:]
                ytp[:, ci * CW:(ci + 1) * CW] = it.view(np.int8)
            else:
                ytp[:, ci * CW:(ci + 1) * CW] = t
        yftp = np.ascontiguousarray(
            yn[:, KI:].astype(np.float16).T)     # [128, CW]
        in_maps.append({"y8": ytp, "yf": yftp, "w": w_ext})

    res = run_bass_kernel_spmd(_get_nc(), in_maps, list(range(NCORES)))
    _CACHE["last_result"] = res

    out = np.empty((B, MC), dtype=np.float32)
    for sh in range(NCORES):
        # ut[32*bc + c, j] = (u^T[c, bc*512 + j] / srow) / 64
        stripes = res.results[sh]["ut"].reshape(NB, 32, NFREE)[:, :MC, :]
        u = stripes.transpose(0, 2, 1).reshape(BS, MC).astype(np.float32)
        out[sh * BS:(sh + 1) * BS, :] = u * (srows[sh] * float(2 ** WSHIFT))[:, None]
    return out


# revision 9
# speedup vs baseline: 1.1546x; 1.0079x over previous
"""Trainium2 Bass kernel for nn_DSC_86071144612259.

The reference network collapses to a single linear contraction

    u[b, c] = sum_{d<128} sum_{p} W[d, p, c] * y_rev[b, d, p]

where W [128, P, MC] is assembled exactly (float64, on host) from the
small parameter tensors.  The 270 MB y_rev stream is the real work and
is purely HBM bound (~0.45 MB/us per-core share on the sync HWDGE ring
Q1), so y moves as 1 byte/element.  The PE only eats float dtypes, so
bytes are widened to fp16 on-chip.  Measured facts driving the design
(all from HW traces on this problem):

* DMA-completion semaphores become visible to waiting engines ~2.9 us
  after the queue finishes (in-flight pipeline), so every DMA-gated
  stage pays that once; engine-to-engine semaphores are fast (~0.2 us).
* Direct DVE tensor_copy int8->fp16 runs 1.14 us/chunk ([128, 2048]),
  ACT activation-Copy 1.89 us/chunk; together (1.4 chunk/us) they are
  SLOWER than the arrival rate (1.7 chunk/us) and become the critical
  chain.  Instead, DVE chunks stream as *biased uint8* (y+128) and are
  widened by two dual-op tensor_scalars on uint16 views:
      lo = (v & 0x00FF) | 0x6400 ;  hi = (v >> 8) | 0x6400
  0x6400 | b is exactly fp16(1024 + b), so the fp16 lane holds
  y + 128 + 1024 exactly -- and the constant 1152*sum(W) per output
  channel is removed by the final PSUM->SBUF copy, fused as a
  per-partition tensor_scalar subtract.  The packed ops hit the DVE
  2x/4x mode: 0.67 us/chunk, 1.7x the direct cast.
* The last k-chunk is sent as fp16 directly (host pre-scales by
  1/s_row) so the tail pays DMA-visibility + matmul only, no cast.

The per-row dequant scale (and 2^6 for the W/64 tile, which keeps the
biased PSUM inside fp32 headroom) is applied on the HOST on the tiny
[B, 16] output.  Sharding: pure data parallel over batch across 8
cores (2048 rows each); W is a replicated per-core input.
"""

import numpy as np

B = 16384      # batch
L = 129        # history length of y_rev
P = 32         # observation dim
MC = 16        # control dim (output)
H = 24         # spectral dim
M = 64         # filter length
NCORES = 8
BS = B // NCORES           # 2048 batch rows per core
KD = 128                   # delays with nonzero weight
K = KD * P                 # 4096 contraction length
NKC = K // 128             # 32 k-chunks of 128 partitions
CW = BS                    # SBUF columns per chunk (2048)
NFREE = 512                # matmul moving free dim (one fp32 PSUM bank)
NB = BS // NFREE           # 4 batch chunks per core

NI8 = 30                   # chunks 0..29 stream as bytes (DVE/ACT widen)
WSHIFT = 6                 # W tile is W / 2^WSHIFT; host multiplies back

# byte-chunk group structure on the sync HWDGE ring: fine at the head
# (widening starts ASAP after the ~2.9 us visibility lag), coarse in
# the middle, fine again at the tail (drain granularity).
I8_GROUPS = [[0], [1, 2], [3, 4], [5, 6, 7], [8, 9, 10, 11],
             [12, 13, 14, 15], [16, 17, 18, 19], [20, 21, 22, 23],
             [24, 25, 26], [27, 28], [29]]

# ACT (activation-Copy, ~2.0-2.5 us/chunk incl. per-group visibility
# gating) only helps mid-stream; the head and tail must drain on the
# 0.67 us/chunk DVE packed path or they gate the PE.
ACT_CHUNKS = (11, 15, 19, 23, 26)

CONV_ENGINES = ("vector", "scalar")

_CACHE = {}


def _conv_runs():
    grp = {}
    for gi, chunks in enumerate(I8_GROUPS):
        for ci in chunks:
            grp[ci] = gi
    return [("scalar" if ci in ACT_CHUNKS else "vector", ci, grp[ci])
            for ci in range(NI8)]


def _dve_chunks():
    return sorted(ci for e, ci, g in _conv_runs() if e == "vector")


def _build_w(M0, M_tilde, M_0l, M_big, sigma, lambda_e, phi, phi_tilde):
    """Collapse the parameter tensors into W [KD, MC, P] (float64).

    Mirrors reference.py exactly:
      term1: delay 0,      M0
      term2: delays 1..64, sum_i lambda_i^0.25 phi_tilde[j-1,i] M_tilde[i]
      term3: delays 0..63, sum_l sigma_l^0.25  phi[k,l]         M_0l[l]
      term4: delays 1..127 via conv(phi_tilde[:,i], phi[:,l]) and M_big
    """
    f8 = np.float64
    M0 = M0.astype(f8)
    M_tilde = M_tilde.astype(f8)
    M_0l = M_0l.astype(f8)
    M_big = M_big.astype(f8)
    sigma = sigma.astype(f8)
    lambda_e = lambda_e.astype(f8)
    phi = phi.astype(f8)
    phi_tilde = phi_tilde.astype(f8)

    W = np.zeros((KD, MC, P), dtype=f8)
    W[0] += M0
    pt = phi_tilde * (lambda_e ** 0.25)[None, :]
    W[1:M + 1] += np.einsum("ji,icp->jcp", pt, M_tilde)
    ps = phi * (sigma ** 0.25)[None, :]
    W[0:M] += np.einsum("kl,lcp->kcp", ps, M_0l)
    W4 = np.empty((H, H, 2 * M - 1), dtype=f8)
    for i in range(H):
        for l in range(H):
            W4[i, l] = np.convolve(phi_tilde[:, i], phi[:, l])
    scale = (lambda_e[:, None] * sigma[None, :]) ** 0.25
    W[1:2 * M] += np.einsum("ild,ilcp->dcp", W4 * scale[:, :, None], M_big)
    return W


def _get_nc():
    """Build the per-core Bass program (cached)."""
    if "nc" in _CACHE:
        return _CACHE["nc"]
    import concourse.bass as bass
    import concourse.mybir as mybir

    runs = _conv_runs()
    # per-chunk: (engine, run-ordinal on that engine) for matmul waits
    chunk_wait = {}
    count = {e: 0 for e in CONV_ENGINES}
    for ename, ci, gi in runs:
        count[ename] += 1
        chunk_wait[ci] = (ename, count[ename])
    assert sorted(chunk_wait) == list(range(NI8))

    nc = bass.Bass("TRN2", target_bir_lowering=False, enable_partition_id=False)
    y8 = nc.dram_tensor("y8", [128, NI8 * CW], mybir.dt.int8, kind="ExternalInput")
    yf = nc.dram_tensor("yf", [128, 2 * CW], mybir.dt.float16,
                        kind="ExternalInput")
    # w columns 0..511: swizzled W/64 fp16; columns 512..513: the fp32
    # bias-correction vector (1152*sum_dve(W)) bit-packed as 2 fp16 cols.
    w = nc.dram_tensor("w", [128, NKC * MC + 2], mybir.dt.float16,
                       kind="ExternalInput")
    ut = nc.dram_tensor("ut", [128, NFREE], mybir.dt.float16, kind="ExternalOutput")

    y8_sb = nc.alloc_sbuf_tensor("y8_sb", [128, NI8 * CW], mybir.dt.int8)
    y_sb = nc.alloc_sbuf_tensor("y_sb", [128, NI8 * CW], mybir.dt.float16)
    yf_sb = nc.alloc_sbuf_tensor("yf_sb", [128, 2 * CW], mybir.dt.float16)
    w_sb = nc.alloc_sbuf_tensor("w_sb", [128, NKC * MC + 2], mybir.dt.float16)
    u_sb = nc.alloc_sbuf_tensor("u_sb", [128, NFREE], mybir.dt.float16)
    warm_sb = nc.alloc_sbuf_tensor("warm_sb", [128, 4], mybir.dt.float16)
    ps = nc.alloc_psum_tensor("ps", [128, NFREE], mybir.dt.float32)

    sem_g = [nc.alloc_semaphore(f"sem_g{g}") for g in range(len(I8_GROUPS))]
    sem_f = nc.alloc_semaphore("sem_f")     # fp16 chunk 31
    sem_w = nc.alloc_semaphore("sem_w")
    sem_cv = {e: nc.alloc_semaphore(f"sem_cv_{e}") for e in CONV_ENGINES}
    pe_done = nc.alloc_semaphore("pe_done")
    out_done = nc.alloc_semaphore("out_done")
    odma = nc.alloc_semaphore("odma")

    corr_ap = w_sb[:, NKC * MC:NKC * MC + 2].bitcast(mybir.dt.float32)

    def conv_ops(eng, ename):
        lastg = None
        for ename_r, ci, gi in runs:
            if ename_r != ename:
                continue
            if gi != lastg:
                eng.wait_ge(sem_g[gi], 16)
                lastg = gi
            lo, hi = ci * CW, (ci + 1) * CW
            if ename == "scalar":
                eng.copy(
                    out=y_sb[:, lo:hi], in_=y8_sb[:, lo:hi]
                ).then_inc(sem_cv[ename], 1)
            else:
                xv = y8_sb[:, lo:hi].bitcast(mybir.dt.uint16)
                lov = y_sb[:, lo:lo + CW // 2].bitcast(mybir.dt.uint16)
                hiv = y_sb[:, lo + CW // 2:hi].bitcast(mybir.dt.uint16)
                eng.tensor_scalar(
                    out=lov, in0=xv, scalar1=0x00FF, scalar2=0x6400,
                    op0=mybir.AluOpType.bitwise_and,
                    op1=mybir.AluOpType.bitwise_or,
                )
                eng.tensor_scalar(
                    out=hiv, in0=xv, scalar1=8, scalar2=0x6400,
                    op0=mybir.AluOpType.logical_shift_right,
                    op1=mybir.AluOpType.bitwise_or,
                ).then_inc(sem_cv[ename], 1)

    with nc.Block() as block:

        @block.sync
        def _(sync):
            for g, chunks in enumerate(I8_GROUPS):
                lo, hi = chunks[0] * CW, (chunks[-1] + 1) * CW
                sync.dma_start(
                    out=y8_sb[:, lo:hi], in_=y8[:, lo:hi]
                ).then_inc(sem_g[g], 16)
            sync.dma_start(out=yf_sb[:, :], in_=yf[:, :]).then_inc(sem_f, 16)
            sync.wait_ge(out_done, 1)
            sync.dma_start(
                out=ut[:, :NFREE // 2], in_=u_sb[:, :NFREE // 2]
            ).then_inc(odma, 16)

        @block.scalar
        def _(scalar):
            # W first (tensor engine blocks on it); then a dummy Copy
            # to pull the ~1.3 us activation-table load out of the
            # first cast's critical path (reads garbage, result unused).
            scalar.dma_start(out=w_sb[:, :], in_=w[:, :]).then_inc(sem_w, 16)
            scalar.copy(out=warm_sb[:, :], in_=y8_sb[:, 0:4])
            conv_ops(scalar, "scalar")
            scalar.wait_ge(out_done, 2)
            scalar.dma_start(
                out=ut[:, NFREE // 2:], in_=u_sb[:, NFREE // 2:]
            ).then_inc(odma, 16)

        @block.tensor
        def _(tensor):
            tensor.wait_ge(sem_w, 16)

            for ci in range(NI8):
                e, n = chunk_wait[ci]
                tensor.wait_ge(sem_cv[e], n)
                for bc in range(NB):
                    tensor.matmul(
                        ps[32 * bc:32 * bc + MC, :],
                        w_sb[:, ci * MC:(ci + 1) * MC],
                        y_sb[:, ci * CW + bc * NFREE:ci * CW + (bc + 1) * NFREE],
                        start=(ci == 0),
                        stop=False,
                        tile_position=(0, 32 * bc),
                    )
            # chunks 30,31 fp16-direct; chunk 31 finishes in N=256
            # halves (h0 first across all bc) so the fused
            # subtract-copies and stores can chase.
            tensor.wait_ge(sem_f, 16)
            for bc in range(NB):
                tensor.matmul(
                    ps[32 * bc:32 * bc + MC, :],
                    w_sb[:, NI8 * MC:(NI8 + 1) * MC],
                    yf_sb[:, bc * NFREE:(bc + 1) * NFREE],
                    start=False,
                    stop=False,
                    tile_position=(0, 32 * bc),
                )
            ci = NI8 + 1
            for half in range(2):
                for bc in range(NB):
                    lo = CW + bc * NFREE + half * (NFREE // 2)
                    hi = lo + NFREE // 2
                    plo, phi_ = half * (NFREE // 2), (half + 1) * (NFREE // 2)
                    tensor.matmul(
                        ps[32 * bc:32 * bc + MC, plo:phi_],
                        w_sb[:, ci * MC:(ci + 1) * MC],
                        yf_sb[:, lo:hi],
                        start=False,
                        stop=True,
                        tile_position=(0, 32 * bc),
                    ).then_inc(pe_done, 1)

        @block.vector
        def _(vector):
            conv_ops(vector, "vector")
            # half-0 matmuls are pe_done incs 1..4
            vector.wait_ge(pe_done, 4)
            vector.tensor_scalar(
                out=u_sb[:, :NFREE // 2], in0=ps[:, :NFREE // 2],
                scalar1=corr_ap, scalar2=None,
                op0=mybir.AluOpType.subtract,
            ).then_inc(out_done, 1)
            vector.wait_ge(pe_done, 8)
            vector.tensor_scalar(
                out=u_sb[:, NFREE // 2:], in0=ps[:, NFREE // 2:],
                scalar1=corr_ap, scalar2=None,
                op0=mybir.AluOpType.subtract,
            ).then_inc(out_done, 1)

    _CACHE["nc"] = nc
    return nc


def _ensure_ntff_hook():
    """bass_utils hard-imports antenv.axon_hooks when BASS_TRACE is set;
    this container's trimmed antenv lacks it.  Register a working stub
    built from trn_agent_boot's ctypes NTFF driver (or a None hook,
    which bass_utils degrades gracefully on)."""
    import importlib.util
    import sys
    import types

    if "antenv.axon_hooks" in sys.modules:
        return
    try:
        if importlib.util.find_spec("antenv.axon_hooks") is not None:
            return
    except (ImportError, ValueError):
        pass
    try:
        from trn_agent_boot.trn_boot import _ntff_profile_via_ctypes

        hook = _ntff_profile_via_ctypes("/opt/axon/libaxon_pjrt.so")
    except Exception:
        hook = None
    mod = types.ModuleType("antenv.axon_hooks")
    mod.get_axon_ntff_profile_hook = lambda: hook
    sys.modules["antenv.axon_hooks"] = mod


def kernel(y_rev, M0, M_tilde, M_0l, M_big, sigma, lambda_e, phi, phi_tilde):
    _ensure_ntff_hook()
    from concourse.bass_utils import run_bass_kernel_spmd

    W = _build_w(M0, M_tilde, M_0l, M_big, sigma, lambda_e, phi, phi_tilde)
    # W_flat[k, c] with k = d*P + p, then swizzled so chunk ki sits at
    # columns [ki*MC, (ki+1)*MC) of a [128, NKC*MC] tile; / 2^WSHIFT
    # keeps PSUM magnitudes inside fp16 range for the output tile.
    Wf = W.transpose(0, 2, 1).reshape(K, MC) * (0.5 ** WSHIFT)
    Wsw = Wf.reshape(NKC, 128, MC).transpose(1, 0, 2).reshape(128, NKC * MC)
    Wd = np.ascontiguousarray(Wsw).astype(np.float16)

    # bias correction: DVE chunks carry y+128, widened to 1024+(y+128) =
    # y + 1152, so PSUM holds u' + 1152*sum_{k in DVE chunks} Wd[k, c].
    dve = _dve_chunks()
    Wd64 = Wd.astype(np.float64)           # the values actually multiplied
    corr = np.zeros(128, dtype=np.float64)  # [32*bc + c] layout, same per bc
    csum = np.zeros(MC, dtype=np.float64)
    for ci in dve:
        csum += 1152.0 * Wd64.reshape(128, NKC, MC)[:, ci, :].sum(axis=0)
    for bc in range(NB):
        corr[32 * bc:32 * bc + MC] = csum
    w_ext = np.empty((128, NKC * MC + 2), dtype=np.float16)
    w_ext[:, :NKC * MC] = Wd
    w_ext[:, NKC * MC:] = (
        corr.astype(np.float32).view(np.float16).reshape(128, 2)
    )

    dve_set = set(dve)
    KI = NI8 * 128            # byte-streamed contraction prefix
    in_maps = []
    srows = []
    for sh in range(NCORES):
        blk = y_rev[sh * BS:(sh + 1) * BS, :KD, :].reshape(BS, K)  # [b, k]
        srow = (np.abs(blk).max(axis=1) / 127.0).astype(np.float32)  # [BS]
        np.maximum(srow, 1e-30, out=srow)
        srows.append(srow)
        yn = blk / srow[:, None]                 # |yn| <= 127
        q = np.rint(yn[:, :KI])
        np.clip(q, -127, 127, out=q)
        q = q.astype(np.int8)
        # partition-major layout per chunk: tile[p, j] = q[j, ki*128 + p];
        # DVE chunks additionally biased +128 and column-interleaved so the
        # packed unpack writes halves [0:1024]=even input cols, [1024:2048].
        ytp = np.empty((128, NI8 * CW), dtype=np.int8)
        qT = q.T.reshape(NI8, 128, CW)           # [ki, p, j]
        for ci in range(NI8):
            t = qT[ci]
            if ci in dve_set:
                bt = (t.astype(np.int16) + 128).astype(np.uint8)
                it = np.empty((128, CW), dtype=np.uint8)
                it[:, 0::2] = bt[:, :CW // 2]
                it[:, 1::2] = bt[:, CW // 2:]
                ytp[:, ci * CW:(ci + 1) * CW] = it.view(np.int8)
            else:
                ytp[:, ci * CW:(ci + 1) * CW] = t
        # fp16-direct chunks 30,31: [128, 2*CW], chunk-major columns
        yftp = np.ascontiguousarray(
            yn[:, KI:].astype(np.float16).T
            .reshape(2, 128, CW).transpose(1, 0, 2).reshape(128, 2 * CW))
        in_maps.append({"y8": ytp, "yf": yftp, "w": w_ext})

    res = run_bass_kernel_spmd(_get_nc(), in_maps, list(range(NCORES)))
    _CACHE["last_result"] = res

    out = np.empty((B, MC), dtype=np.float32)
    for sh in range(NCORES):
        # ut[32*bc + c, j] = (u^T[c, bc*512 + j] / srow) / 64
        stripes = res.results[sh]["ut"].reshape(NB, 32, NFREE)[:, :MC, :]
        u = stripes.transpose(0, 2, 1).reshape(BS, MC).astype(np.float32)
        out[sh * BS:(sh + 1) * BS, :] = u * (srows[sh] * float(2 ** WSHIFT))[:, None]
    return out


# revision 11
# speedup vs baseline: 1.1573x; 1.0024x over previous
"""Trainium2 Bass kernel for nn_DSC_86071144612259.

The reference network collapses to a single linear contraction

    u[b, c] = sum_{d<128} sum_{p} W[d, p, c] * y_rev[b, d, p]

where W [128, P, MC] is assembled exactly (float64, on host) from the
small parameter tensors.  The 270 MB y_rev stream is the real work and
is purely HBM bound (~0.45 MB/us per-core share on the sync HWDGE ring
Q1), so y moves as 1 byte/element.  The PE only eats float dtypes, so
bytes are widened to fp16 on-chip.  Measured facts driving the design
(all from HW traces on this problem):

* DMA-completion semaphores become visible to waiting engines ~2.9 us
  after the queue finishes (in-flight pipeline), so every DMA-gated
  stage pays that once; engine-to-engine semaphores are fast (~0.2 us).
* Direct DVE tensor_copy int8->fp16 runs 1.14 us/chunk ([128, 2048]),
  ACT activation-Copy 1.89 us/chunk; together (1.4 chunk/us) they are
  SLOWER than the arrival rate (1.7 chunk/us) and become the critical
  chain.  Instead, DVE chunks stream as *biased uint8* (y+128) and are
  widened by two dual-op tensor_scalars on uint16 views:
      lo = (v & 0x00FF) | 0x6400 ;  hi = (v >> 8) | 0x6400
  0x6400 | b is exactly fp16(1024 + b), so the fp16 lane holds
  y + 128 + 1024 exactly -- and the constant 1152*sum(W) per output
  channel is removed by the final PSUM->SBUF copy, fused as a
  per-partition tensor_scalar subtract.  The packed ops hit the DVE
  2x/4x mode: 0.67 us/chunk, 1.7x the direct cast.
* The last k-chunk is sent as fp16 directly (host pre-scales by
  1/s_row) so the tail pays DMA-visibility + matmul only, no cast.

The per-row dequant scale (and 2^6 for the W/64 tile, which keeps the
biased PSUM inside fp32 headroom) is applied on the HOST on the tiny
[B, 16] output.  Sharding: pure data parallel over batch across 8
cores (2048 rows each); W is a replicated per-core input.
"""

import numpy as np

B = 16384      # batch
L = 129        # history length of y_rev
P = 32         # observation dim
MC = 16        # control dim (output)
H = 24         # spectral dim
M = 64         # filter length
NCORES = 8
BS = B // NCORES           # 2048 batch rows per core
KD = 128                   # delays with nonzero weight
K = KD * P                 # 4096 contraction length
NKC = K // 128             # 32 k-chunks of 128 partitions
CW = BS                    # SBUF columns per chunk (2048)
NFREE = 512                # matmul moving free dim (one fp32 PSUM bank)
NB = BS // NFREE           # 4 batch chunks per core

NI8 = 30                   # chunks 0..29 stream as bytes (DVE/ACT widen)
WSHIFT = 6                 # W tile is W / 2^WSHIFT; host multiplies back

# byte-chunk group structure on the sync HWDGE ring: fine at the head
# (widening starts ASAP after the ~2.9 us visibility lag), coarse in
# the middle, fine again at the tail (drain granularity).
I8_GROUPS = [[0], [1, 2], [3, 4], [5, 6, 7], [8, 9, 10, 11],
             [12, 13, 14, 15], [16, 17, 18, 19], [20, 21, 22, 23],
             [24, 25, 26], [27, 28], [29]]

# ACT (activation-Copy, ~2.0-2.5 us/chunk incl. per-group visibility
# gating) only helps mid-stream; the head and tail must drain on the
# 0.67 us/chunk DVE packed path or they gate the PE.
ACT_CHUNKS = (3, 11, 15, 19, 23, 26)

CONV_ENGINES = ("vector", "scalar")

_CACHE = {}


def _conv_runs():
    grp = {}
    for gi, chunks in enumerate(I8_GROUPS):
        for ci in chunks:
            grp[ci] = gi
    return [("scalar" if ci in ACT_CHUNKS else "vector", ci, grp[ci])
            for ci in range(NI8)]


def _dve_chunks():
    return sorted(ci for e, ci, g in _conv_runs() if e == "vector")


def _build_w(M0, M_tilde, M_0l, M_big, sigma, lambda_e, phi, phi_tilde):
    """Collapse the parameter tensors into W [KD, MC, P] (float64).

    Mirrors reference.py exactly:
      term1: delay 0,      M0
      term2: delays 1..64, sum_i lambda_i^0.25 phi_tilde[j-1,i] M_tilde[i]
      term3: delays 0..63, sum_l sigma_l^0.25  phi[k,l]         M_0l[l]
      term4: delays 1..127 via conv(phi_tilde[:,i], phi[:,l]) and M_big
    """
    f8 = np.float64
    M0 = M0.astype(f8)
    M_tilde = M_tilde.astype(f8)
    M_0l = M_0l.astype(f8)
    M_big = M_big.astype(f8)
    sigma = sigma.astype(f8)
    lambda_e = lambda_e.astype(f8)
    phi = phi.astype(f8)
    phi_tilde = phi_tilde.astype(f8)

    W = np.zeros((KD, MC, P), dtype=f8)
    W[0] += M0
    pt = phi_tilde * (lambda_e ** 0.25)[None, :]
    W[1:M + 1] += np.einsum("ji,icp->jcp", pt, M_tilde)
    ps = phi * (sigma ** 0.25)[None, :]
    W[0:M] += np.einsum("kl,lcp->kcp", ps, M_0l)
    W4 = np.empty((H, H, 2 * M - 1), dtype=f8)
    for i in range(H):
        for l in range(H):
            W4[i, l] = np.convolve(phi_tilde[:, i], phi[:, l])
    scale = (lambda_e[:, None] * sigma[None, :]) ** 0.25
    W[1:2 * M] += np.einsum("ild,ilcp->dcp", W4 * scale[:, :, None], M_big)
    return W


def _get_nc():
    """Build the per-core Bass program (cached)."""
    if "nc" in _CACHE:
        return _CACHE["nc"]
    import concourse.bass as bass
    import concourse.mybir as mybir

    runs = _conv_runs()
    # per-chunk: (engine, run-ordinal on that engine) for matmul waits
    chunk_wait = {}
    count = {e: 0 for e in CONV_ENGINES}
    for ename, ci, gi in runs:
        count[ename] += 1
        chunk_wait[ci] = (ename, count[ename])
    assert sorted(chunk_wait) == list(range(NI8))

    nc = bass.Bass("TRN2", target_bir_lowering=False, enable_partition_id=False)
    y8 = nc.dram_tensor("y8", [128, NI8 * CW], mybir.dt.int8, kind="ExternalInput")
    yf = nc.dram_tensor("yf", [128, 2 * CW], mybir.dt.float16,
                        kind="ExternalInput")
    # w columns 0..511: swizzled W/64 fp16; columns 512..513: the fp32
    # bias-correction vector (1152*sum_dve(W)) bit-packed as 2 fp16 cols.
    w = nc.dram_tensor("w", [128, NKC * MC + 2], mybir.dt.float16,
                       kind="ExternalInput")
    ut = nc.dram_tensor("ut", [128, NFREE], mybir.dt.float16, kind="ExternalOutput")

    y8_sb = nc.alloc_sbuf_tensor("y8_sb", [128, NI8 * CW], mybir.dt.int8)
    y_sb = nc.alloc_sbuf_tensor("y_sb", [128, NI8 * CW], mybir.dt.float16)
    yf_sb = nc.alloc_sbuf_tensor("yf_sb", [128, 2 * CW], mybir.dt.float16)
    w_sb = nc.alloc_sbuf_tensor("w_sb", [128, NKC * MC + 2], mybir.dt.float16)
    u_sb = nc.alloc_sbuf_tensor("u_sb", [128, NFREE], mybir.dt.float16)
    warm_sb = nc.alloc_sbuf_tensor("warm_sb", [128, 4], mybir.dt.float16)
    wake_sb = nc.alloc_sbuf_tensor("wake_sb", [128, 64], mybir.dt.int8)
    ps = nc.alloc_psum_tensor("ps", [128, NFREE], mybir.dt.float32)

    sem_g = [nc.alloc_semaphore(f"sem_g{g}") for g in range(len(I8_GROUPS))]
    sem_f = nc.alloc_semaphore("sem_f")     # fp16 chunk 31
    sem_w = nc.alloc_semaphore("sem_w")
    sem_cv = {e: nc.alloc_semaphore(f"sem_cv_{e}") for e in CONV_ENGINES}
    pe_done = nc.alloc_semaphore("pe_done")
    out_done = nc.alloc_semaphore("out_done")
    odma = nc.alloc_semaphore("odma")

    corr_ap = w_sb[:, NKC * MC:NKC * MC + 2].bitcast(mybir.dt.float32)

    def conv_ops(eng, ename):
        lastg = None
        for ename_r, ci, gi in runs:
            if ename_r != ename:
                continue
            if gi != lastg:
                eng.wait_ge(sem_g[gi], 16)
                lastg = gi
            lo, hi = ci * CW, (ci + 1) * CW
            if ename == "scalar":
                eng.copy(
                    out=y_sb[:, lo:hi], in_=y8_sb[:, lo:hi]
                ).then_inc(sem_cv[ename], 1)
            else:
                xv = y8_sb[:, lo:hi].bitcast(mybir.dt.uint16)
                lov = y_sb[:, lo:lo + CW // 2].bitcast(mybir.dt.uint16)
                hiv = y_sb[:, lo + CW // 2:hi].bitcast(mybir.dt.uint16)
                eng.tensor_scalar(
                    out=lov, in0=xv, scalar1=0x00FF, scalar2=0x6400,
                    op0=mybir.AluOpType.bitwise_and,
                    op1=mybir.AluOpType.bitwise_or,
                )
                eng.tensor_scalar(
                    out=hiv, in0=xv, scalar1=8, scalar2=0x6400,
                    op0=mybir.AluOpType.logical_shift_right,
                    op1=mybir.AluOpType.bitwise_or,
                ).then_inc(sem_cv[ename], 1)

    with nc.Block(no_gpsimd_drain=True) as block:

        @block.sync
        def _(sync):
            # wake all 16 DMA queue engines before the real stream: 128
            # tiny descriptors spread round-robin; result unused.
            sync.dma_start(out=wake_sb[:, :], in_=y8[:, 0:64]).then_inc(odma, 16)
            for g, chunks in enumerate(I8_GROUPS):
                lo, hi = chunks[0] * CW, (chunks[-1] + 1) * CW
                sync.dma_start(
                    out=y8_sb[:, lo:hi], in_=y8[:, lo:hi]
                ).then_inc(sem_g[g], 16)
            sync.dma_start(out=yf_sb[:, :], in_=yf[:, :]).then_inc(sem_f, 16)
            sync.wait_ge(out_done, 1)
            sync.dma_start(
                out=ut[:, :NFREE // 2], in_=u_sb[:, :NFREE // 2]
            ).then_inc(odma, 16)

        @block.scalar
        def _(scalar):
            # W first (tensor engine blocks on it); then a dummy Copy
            # to pull the ~1.3 us activation-table load out of the
            # first cast's critical path (reads garbage, result unused).
            scalar.dma_start(out=w_sb[:, :], in_=w[:, :]).then_inc(sem_w, 16)
            scalar.copy(out=warm_sb[:, :], in_=y8_sb[:, 0:4])
            conv_ops(scalar, "scalar")
            scalar.wait_ge(out_done, 2)
            scalar.dma_start(
                out=ut[:, NFREE // 2:], in_=u_sb[:, NFREE // 2:]
            ).then_inc(odma, 16)

        @block.tensor
        def _(tensor):
            tensor.wait_ge(sem_w, 16)

            for ci in range(NI8):
                e, n = chunk_wait[ci]
                tensor.wait_ge(sem_cv[e], n)
                for bc in range(NB):
                    tensor.matmul(
                        ps[32 * bc:32 * bc + MC, :],
                        w_sb[:, ci * MC:(ci + 1) * MC],
                        y_sb[:, ci * CW + bc * NFREE:ci * CW + (bc + 1) * NFREE],
                        start=(ci == 0),
                        stop=False,
                        tile_position=(0, 32 * bc),
                    )
            # chunks 30,31 fp16-direct; chunk 31 finishes in N=256
            # halves (h0 first across all bc) so the fused
            # subtract-copies and stores can chase.
            tensor.wait_ge(sem_f, 16)
            for bc in range(NB):
                tensor.matmul(
                    ps[32 * bc:32 * bc + MC, :],
                    w_sb[:, NI8 * MC:(NI8 + 1) * MC],
                    yf_sb[:, bc * NFREE:(bc + 1) * NFREE],
                    start=False,
                    stop=False,
                    tile_position=(0, 32 * bc),
                )
            ci = NI8 + 1
            for half in range(2):
                for bc in range(NB):
                    lo = CW + bc * NFREE + half * (NFREE // 2)
                    hi = lo + NFREE // 2
                    plo, phi_ = half * (NFREE // 2), (half + 1) * (NFREE // 2)
                    tensor.matmul(
                        ps[32 * bc:32 * bc + MC, plo:phi_],
                        w_sb[:, ci * MC:(ci + 1) * MC],
                        yf_sb[:, lo:hi],
                        start=False,
                        stop=True,
                        tile_position=(0, 32 * bc),
                    ).then_inc(pe_done, 1)

        @block.vector
        def _(vector):
            conv_ops(vector, "vector")
            # half-0 matmuls are pe_done incs 1..4
            vector.wait_ge(pe_done, 4)
            vector.tensor_scalar(
                out=u_sb[:, :NFREE // 2], in0=ps[:, :NFREE // 2],
                scalar1=corr_ap, scalar2=None,
                op0=mybir.AluOpType.subtract,
            ).then_inc(out_done, 1)
            vector.wait_ge(pe_done, 8)
            vector.tensor_scalar(
                out=u_sb[:, NFREE // 2:], in0=ps[:, NFREE // 2:],
                scalar1=corr_ap, scalar2=None,
                op0=mybir.AluOpType.subtract,
            ).then_inc(out_done, 1)

    _CACHE["nc"] = nc
    return nc


def _ensure_ntff_hook():
    """bass_utils hard-imports antenv.axon_hooks when BASS_TRACE is set;
    this container's trimmed antenv lacks it.  Register a working stub
    built from trn_agent_boot's ctypes NTFF driver (or a None hook,
    which bass_utils degrades gracefully on)."""
    import importlib.util
    import sys
    import types

    if "antenv.axon_hooks" in sys.modules:
        return
    try:
        if importlib.util.find_spec("antenv.axon_hooks") is not None:
            return
    except (ImportError, ValueError):
        pass
    try:
        from trn_agent_boot.trn_boot import _ntff_profile_via_ctypes

        hook = _ntff_profile_via_ctypes("/opt/axon/libaxon_pjrt.so")
    except Exception:
        hook = None
    mod = types.ModuleType("antenv.axon_hooks")
    mod.get_axon_ntff_profile_hook = lambda: hook
    sys.modules["antenv.axon_hooks"] = mod


def kernel(y_rev, M0, M_tilde, M_0l, M_big, sigma, lambda_e, phi, phi_tilde):
    _ensure_ntff_hook()
    from concourse.bass_utils import run_bass_kernel_spmd

    W = _build_w(M0, M_tilde, M_0l, M_big, sigma, lambda_e, phi, phi_tilde)
    # W_flat[k, c] with k = d*P + p, then swizzled so chunk ki sits at
    # columns [ki*MC, (ki+1)*MC) of a [128, NKC*MC] tile; / 2^WSHIFT
    # keeps PSUM magnitudes inside fp16 range for the output tile.
    Wf = W.transpose(0, 2, 1).reshape(K, MC) * (0.5 ** WSHIFT)
    Wsw = Wf.reshape(NKC, 128, MC).transpose(1, 0, 2).reshape(128, NKC * MC)
    Wd = np.ascontiguousarray(Wsw).astype(np.float16)

    # bias correction: DVE chunks carry y+128, widened to 1024+(y+128) =
    # y + 1152, so PSUM holds u' + 1152*sum_{k in DVE chunks} Wd[k, c].
    dve = _dve_chunks()
    Wd64 = Wd.astype(np.float64)           # the values actually multiplied
    corr = np.zeros(128, dtype=np.float64)  # [32*bc + c] layout, same per bc
    csum = np.zeros(MC, dtype=np.float64)
    for ci in dve:
        csum += 1152.0 * Wd64.reshape(128, NKC, MC)[:, ci, :].sum(axis=0)
    for bc in range(NB):
        corr[32 * bc:32 * bc + MC] = csum
    w_ext = np.empty((128, NKC * MC + 2), dtype=np.float16)
    w_ext[:, :NKC * MC] = Wd
    w_ext[:, NKC * MC:] = (
        corr.astype(np.float32).view(np.float16).reshape(128, 2)
    )

    dve_set = set(dve)
    KI = NI8 * 128            # byte-streamed contraction prefix
    in_maps = []
    srows = []
    for sh in range(NCORES):
        blk = y_rev[sh * BS:(sh + 1) * BS, :KD, :].reshape(BS, K)  # [b, k]
        srow = (np.abs(blk).max(axis=1) / 127.0).astype(np.float32)  # [BS]
        np.maximum(srow, 1e-30, out=srow)
        srows.append(srow)
        yn = blk / srow[:, None]                 # |yn| <= 127
        q = np.rint(yn[:, :KI])
        np.clip(q, -127, 127, out=q)
        q = q.astype(np.int8)
        # partition-major layout per chunk: tile[p, j] = q[j, ki*128 + p];
        # DVE chunks additionally biased +128 and column-interleaved so the
        # packed unpack writes halves [0:1024]=even input cols, [1024:2048].
        ytp = np.empty((128, NI8 * CW), dtype=np.int8)
        qT = q.T.reshape(NI8, 128, CW)           # [ki, p, j]
        for ci in range(NI8):
            t = qT[ci]
            if ci in dve_set:
                bt = (t.astype(np.int16) + 128).astype(np.uint8)
                it = np.empty((128, CW), dtype=np.uint8)
                it[:, 0::2] = bt[:, :CW // 2]
                it[:, 1::2] = bt[:, CW // 2:]
                ytp[:, ci * CW:(ci + 1) * CW] = it.view(np.int8)
            else:
                ytp[:, ci * CW:(ci + 1) * CW] = t
        # fp16-direct chunks 30,31: [128, 2*CW], chunk-major columns
        yftp = np.ascontiguousarray(
            yn[:, KI:].astype(np.float16).T
            .reshape(2, 128, CW).transpose(1, 0, 2).reshape(128, 2 * CW))
        in_maps.append({"y8": ytp, "yf": yftp, "w": w_ext})

    res = run_bass_kernel_spmd(_get_nc(), in_maps, list(range(NCORES)))
    _CACHE["last_result"] = res

    out = np.empty((B, MC), dtype=np.float32)
    for sh in range(NCORES):
        # ut[32*bc + c, j] = (u^T[c, bc*512 + j] / srow) / 64
        stripes = res.results[sh]["ut"].reshape(NB, 32, NFREE)[:, :MC, :]
        u = stripes.transpose(0, 2, 1).reshape(BS, MC).astype(np.float32)
        out[sh * BS:(sh + 1) * BS, :] = u * (srows[sh] * float(2 ** WSHIFT))[:, None]
    return out


# revision 12
# speedup vs baseline: 1.1624x; 1.0044x over previous
"""Trainium2 Bass kernel for nn_DSC_86071144612259.

The reference network collapses to a single linear contraction

    u[b, c] = sum_{d<128} sum_{p} W[d, p, c] * y_rev[b, d, p]

where W [128, P, MC] is assembled exactly (float64, on host) from the
small parameter tensors.  The 270 MB y_rev stream is the real work and
is purely HBM bound (~0.45 MB/us per-core share on the sync HWDGE ring
Q1), so y moves as 1 byte/element.  The PE only eats float dtypes, so
bytes are widened to fp16 on-chip.  Measured facts driving the design
(all from HW traces on this problem):

* DMA-completion semaphores become visible to waiting engines ~2.9 us
  after the queue finishes (in-flight pipeline), so every DMA-gated
  stage pays that once; engine-to-engine semaphores are fast (~0.2 us).
* Direct DVE tensor_copy int8->fp16 runs 1.14 us/chunk ([128, 2048]),
  ACT activation-Copy 1.89 us/chunk; together (1.4 chunk/us) they are
  SLOWER than the arrival rate (1.7 chunk/us) and become the critical
  chain.  Instead, DVE chunks stream as *biased uint8* (y+128) and are
  widened by two dual-op tensor_scalars on uint16 views:
      lo = (v & 0x00FF) | 0x6400 ;  hi = (v >> 8) | 0x6400
  0x6400 | b is exactly fp16(1024 + b), so the fp16 lane holds
  y + 128 + 1024 exactly -- and the constant 1152*sum(W) per output
  channel is removed by the final PSUM->SBUF copy, fused as a
  per-partition tensor_scalar subtract.  The packed ops hit the DVE
  2x/4x mode: 0.67 us/chunk, 1.7x the direct cast.
* The last k-chunk is sent as fp16 directly (host pre-scales by
  1/s_row) so the tail pays DMA-visibility + matmul only, no cast.

The per-row dequant scale (and 2^6 for the W/64 tile, which keeps the
biased PSUM inside fp32 headroom) is applied on the HOST on the tiny
[B, 16] output.  Sharding: pure data parallel over batch across 8
cores (2048 rows each); W is a replicated per-core input.
"""

import numpy as np

B = 16384      # batch
L = 129        # history length of y_rev
P = 32         # observation dim
MC = 16        # control dim (output)
H = 24         # spectral dim
M = 64         # filter length
NCORES = 8
BS = B // NCORES           # 2048 batch rows per core
KD = 128                   # delays with nonzero weight
K = KD * P                 # 4096 contraction length
NKC = K // 128             # 32 k-chunks of 128 partitions
CW = BS                    # SBUF columns per chunk (2048)
NFREE = 512                # matmul moving free dim (one fp32 PSUM bank)
NB = BS // NFREE           # 4 batch chunks per core

NI8 = 30                   # chunks 0..29 stream as bytes (DVE/ACT widen)
WSHIFT = 6                 # W tile is W / 2^WSHIFT; host multiplies back

# byte-chunk group structure on the sync HWDGE ring: fine at the head
# (widening starts ASAP after the ~2.9 us visibility lag), coarse in
# the middle, fine again at the tail (drain granularity).
I8_GROUPS = [[0], [1, 2], [3, 4], [5, 6, 7], [8, 9, 10, 11],
             [12, 13, 14, 15], [16, 17, 18, 19], [20, 21, 22, 23],
             [24, 25, 26], [27, 28], [29]]

# ACT (activation-Copy, ~2.0-2.5 us/chunk incl. per-group visibility
# gating) only helps mid-stream; the head and tail must drain on the
# 0.67 us/chunk DVE packed path or they gate the PE.
ACT_CHUNKS = (3, 7, 11, 14, 17, 20, 24)

CONV_ENGINES = ("vector", "scalar")

_CACHE = {}


def _conv_runs():
    grp = {}
    for gi, chunks in enumerate(I8_GROUPS):
        for ci in chunks:
            grp[ci] = gi
    return [("scalar" if ci in ACT_CHUNKS else "vector", ci, grp[ci])
            for ci in range(NI8)]


def _dve_chunks():
    return sorted(ci for e, ci, g in _conv_runs() if e == "vector")


def _build_w(M0, M_tilde, M_0l, M_big, sigma, lambda_e, phi, phi_tilde):
    """Collapse the parameter tensors into W [KD, MC, P] (float64).

    Mirrors reference.py exactly:
      term1: delay 0,      M0
      term2: delays 1..64, sum_i lambda_i^0.25 phi_tilde[j-1,i] M_tilde[i]
      term3: delays 0..63, sum_l sigma_l^0.25  phi[k,l]         M_0l[l]
      term4: delays 1..127 via conv(phi_tilde[:,i], phi[:,l]) and M_big
    """
    f8 = np.float64
    M0 = M0.astype(f8)
    M_tilde = M_tilde.astype(f8)
    M_0l = M_0l.astype(f8)
    M_big = M_big.astype(f8)
    sigma = sigma.astype(f8)
    lambda_e = lambda_e.astype(f8)
    phi = phi.astype(f8)
    phi_tilde = phi_tilde.astype(f8)

    W = np.zeros((KD, MC, P), dtype=f8)
    W[0] += M0
    pt = phi_tilde * (lambda_e ** 0.25)[None, :]
    W[1:M + 1] += np.einsum("ji,icp->jcp", pt, M_tilde)
    ps = phi * (sigma ** 0.25)[None, :]
    W[0:M] += np.einsum("kl,lcp->kcp", ps, M_0l)
    W4 = np.empty((H, H, 2 * M - 1), dtype=f8)
    for i in range(H):
        for l in range(H):
            W4[i, l] = np.convolve(phi_tilde[:, i], phi[:, l])
    scale = (lambda_e[:, None] * sigma[None, :]) ** 0.25
    W[1:2 * M] += np.einsum("ild,ilcp->dcp", W4 * scale[:, :, None], M_big)
    return W


def _get_nc():
    """Build the per-core Bass program (cached)."""
    if "nc" in _CACHE:
        return _CACHE["nc"]
    import concourse.bass as bass
    import concourse.mybir as mybir

    runs = _conv_runs()
    # per-chunk: (engine, run-ordinal on that engine) for matmul waits
    chunk_wait = {}
    count = {e: 0 for e in CONV_ENGINES}
    for ename, ci, gi in runs:
        count[ename] += 1
        chunk_wait[ci] = (ename, count[ename])
    assert sorted(chunk_wait) == list(range(NI8))

    nc = bass.Bass("TRN2", target_bir_lowering=False, enable_partition_id=False)
    y8 = nc.dram_tensor("y8", [128, NI8 * CW], mybir.dt.int8, kind="ExternalInput")
    yf = nc.dram_tensor("yf", [128, 2 * CW], mybir.dt.float16,
                        kind="ExternalInput")
    # w columns 0..511: swizzled W/64 fp16; columns 512..513: the fp32
    # bias-correction vector (1152*sum_dve(W)) bit-packed as 2 fp16 cols.
    w = nc.dram_tensor("w", [128, NKC * MC + 2], mybir.dt.float16,
                       kind="ExternalInput")
    ut = nc.dram_tensor("ut", [128, NFREE], mybir.dt.float16, kind="ExternalOutput")

    y8_sb = nc.alloc_sbuf_tensor("y8_sb", [128, NI8 * CW], mybir.dt.int8)
    y_sb = nc.alloc_sbuf_tensor("y_sb", [128, NI8 * CW], mybir.dt.float16)
    yf_sb = nc.alloc_sbuf_tensor("yf_sb", [128, 2 * CW], mybir.dt.float16)
    w_sb = nc.alloc_sbuf_tensor("w_sb", [128, NKC * MC + 2], mybir.dt.float16)
    u_sb = nc.alloc_sbuf_tensor("u_sb", [128, NFREE], mybir.dt.float16)
    warm_sb = nc.alloc_sbuf_tensor("warm_sb", [128, 4], mybir.dt.float16)
    ps = nc.alloc_psum_tensor("ps", [128, NFREE], mybir.dt.float32)

    sem_g = [nc.alloc_semaphore(f"sem_g{g}") for g in range(len(I8_GROUPS))]
    sem_f = nc.alloc_semaphore("sem_f")     # fp16 chunk 31
    sem_w = nc.alloc_semaphore("sem_w")
    sem_cv = {e: nc.alloc_semaphore(f"sem_cv_{e}") for e in CONV_ENGINES}
    pe_done = nc.alloc_semaphore("pe_done")
    out_done = nc.alloc_semaphore("out_done")
    odma = nc.alloc_semaphore("odma")

    corr_ap = w_sb[:, NKC * MC:NKC * MC + 2].bitcast(mybir.dt.float32)

    def conv_ops(eng, ename):
        lastg = None
        for ename_r, ci, gi in runs:
            if ename_r != ename:
                continue
            if gi != lastg:
                eng.wait_ge(sem_g[gi], 16)
                lastg = gi
            lo, hi = ci * CW, (ci + 1) * CW
            if ename == "scalar":
                eng.copy(
                    out=y_sb[:, lo:hi], in_=y8_sb[:, lo:hi]
                ).then_inc(sem_cv[ename], 1)
            else:
                xv = y8_sb[:, lo:hi].bitcast(mybir.dt.uint16)
                lov = y_sb[:, lo:lo + CW // 2].bitcast(mybir.dt.uint16)
                hiv = y_sb[:, lo + CW // 2:hi].bitcast(mybir.dt.uint16)
                eng.tensor_scalar(
                    out=lov, in0=xv, scalar1=0x00FF, scalar2=0x6400,
                    op0=mybir.AluOpType.bitwise_and,
                    op1=mybir.AluOpType.bitwise_or,
                )
                eng.tensor_scalar(
                    out=hiv, in0=xv, scalar1=8, scalar2=0x6400,
                    op0=mybir.AluOpType.logical_shift_right,
                    op1=mybir.AluOpType.bitwise_or,
                ).then_inc(sem_cv[ename], 1)

    with nc.Block(no_gpsimd_drain=True) as block:

        @block.sync
        def _(sync):
            for g, chunks in enumerate(I8_GROUPS):
                lo, hi = chunks[0] * CW, (chunks[-1] + 1) * CW
                sync.dma_start(
                    out=y8_sb[:, lo:hi], in_=y8[:, lo:hi]
                ).then_inc(sem_g[g], 16)
            sync.dma_start(out=yf_sb[:, :], in_=yf[:, :]).then_inc(sem_f, 16)
            sync.wait_ge(out_done, 1)
            sync.dma_start(
                out=ut[:, :NFREE // 2], in_=u_sb[:, :NFREE // 2]
            ).then_inc(odma, 16)

        @block.scalar
        def _(scalar):
            # W first (tensor engine blocks on it); then a dummy Copy
            # to pull the ~1.3 us activation-table load out of the
            # first cast's critical path (reads garbage, result unused).
            scalar.dma_start(out=w_sb[:, :], in_=w[:, :]).then_inc(sem_w, 16)
            scalar.copy(out=warm_sb[:, :], in_=y8_sb[:, 0:4])
            conv_ops(scalar, "scalar")
            scalar.wait_ge(out_done, 2)
            scalar.dma_start(
                out=ut[:, NFREE // 2:], in_=u_sb[:, NFREE // 2:]
            ).then_inc(odma, 16)

        @block.tensor
        def _(tensor):
            tensor.wait_ge(sem_w, 16)

            for ci in range(NI8):
                e, n = chunk_wait[ci]
                tensor.wait_ge(sem_cv[e], n)
                for bc in range(NB):
                    tensor.matmul(
                        ps[32 * bc:32 * bc + MC, :],
                        w_sb[:, ci * MC:(ci + 1) * MC],
                        y_sb[:, ci * CW + bc * NFREE:ci * CW + (bc + 1) * NFREE],
                        start=(ci == 0),
                        stop=False,
                        tile_position=(0, 32 * bc),
                    )
            # chunks 30,31 fp16-direct; chunk 31 finishes in N=256
            # halves (h0 first across all bc) so the fused
            # subtract-copies and stores can chase.
            tensor.wait_ge(sem_f, 16)
            for bc in range(NB):
                tensor.matmul(
                    ps[32 * bc:32 * bc + MC, :],
                    w_sb[:, NI8 * MC:(NI8 + 1) * MC],
                    yf_sb[:, bc * NFREE:(bc + 1) * NFREE],
                    start=False,
                    stop=False,
                    tile_position=(0, 32 * bc),
                )
            ci = NI8 + 1
            for half in range(2):
                for bc in range(NB):
                    lo = CW + bc * NFREE + half * (NFREE // 2)
                    hi = lo + NFREE // 2
                    plo, phi_ = half * (NFREE // 2), (half + 1) * (NFREE // 2)
                    tensor.matmul(
                        ps[32 * bc:32 * bc + MC, plo:phi_],
                        w_sb[:, ci * MC:(ci + 1) * MC],
                        yf_sb[:, lo:hi],
                        start=False,
                        stop=True,
                        tile_position=(0, 32 * bc),
                    ).then_inc(pe_done, 1)

        @block.vector
        def _(vector):
            conv_ops(vector, "vector")
            # half-0 matmuls are pe_done incs 1..4
            vector.wait_ge(pe_done, 4)
            vector.tensor_scalar(
                out=u_sb[:, :NFREE // 2], in0=ps[:, :NFREE // 2],
                scalar1=corr_ap, scalar2=None,
                op0=mybir.AluOpType.subtract,
            ).then_inc(out_done, 1)
            vector.wait_ge(pe_done, 8)
            vector.tensor_scalar(
                out=u_sb[:, NFREE // 2:], in0=ps[:, NFREE // 2:],
                scalar1=corr_ap, scalar2=None,
                op0=mybir.AluOpType.subtract,
            ).then_inc(out_done, 1)

    _CACHE["nc"] = nc
    return nc


def _ensure_ntff_hook():
    """bass_utils hard-imports antenv.axon_hooks when BASS_TRACE is set;
    this container's trimmed antenv lacks it.  Register a working stub
    built from trn_agent_boot's ctypes NTFF driver (or a None hook,
    which bass_utils degrades gracefully on)."""
    import importlib.util
    import sys
    import types

    if "antenv.axon_hooks" in sys.modules:
        return
    try:
        if importlib.util.find_spec("antenv.axon_hooks") is not None:
            return
    except (ImportError, ValueError):
        pass
    try:
        from trn_agent_boot.trn_boot import _ntff_profile_via_ctypes

        hook = _ntff_profile_via_ctypes("/opt/axon/libaxon_pjrt.so")
    except Exception:
        hook = None
    mod = types.ModuleType("antenv.axon_hooks")
    mod.get_axon_ntff_profile_hook = lambda: hook
    sys.modules["antenv.axon_hooks"] = mod


def kernel(y_rev, M0, M_tilde, M_0l, M_big, sigma, lambda_e, phi, phi_tilde):
    _ensure_ntff_hook()
    from concourse.bass_utils import run_bass_kernel_spmd

    W = _build_w(M0, M_tilde, M_0l, M_big, sigma, lambda_e, phi, phi_tilde)
    # W_flat[k, c] with k = d*P + p, then swizzled so chunk ki sits at
    # columns [ki*MC, (ki+1)*MC) of a [128, NKC*MC] tile; / 2^WSHIFT
    # keeps PSUM magnitudes inside fp16 range for the output tile.
    Wf = W.transpose(0, 2, 1).reshape(K, MC) * (0.5 ** WSHIFT)
    Wsw = Wf.reshape(NKC, 128, MC).transpose(1, 0, 2).reshape(128, NKC * MC)
    Wd = np.ascontiguousarray(Wsw).astype(np.float16)

    # bias correction: DVE chunks carry y+128, widened to 1024+(y+128) =
    # y + 1152, so PSUM holds u' + 1152*sum_{k in DVE chunks} Wd[k, c].
    dve = _dve_chunks()
    Wd64 = Wd.astype(np.float64)           # the values actually multiplied
    corr = np.zeros(128, dtype=np.float64)  # [32*bc + c] layout, same per bc
    csum = np.zeros(MC, dtype=np.float64)
    for ci in dve:
        csum += 1152.0 * Wd64.reshape(128, NKC, MC)[:, ci, :].sum(axis=0)
    for bc in range(NB):
        corr[32 * bc:32 * bc + MC] = csum
    w_ext = np.empty((128, NKC * MC + 2), dtype=np.float16)
    w_ext[:, :NKC * MC] = Wd
    w_ext[:, NKC * MC:] = (
        corr.astype(np.float32).view(np.float16).reshape(128, 2)
    )

    dve_set = set(dve)
    KI = NI8 * 128            # byte-streamed contraction prefix
    in_maps = []
    srows = []
    for sh in range(NCORES):
        blk = y_rev[sh * BS:(sh + 1) * BS, :KD, :].reshape(BS, K)  # [b, k]
        srow = (np.abs(blk).max(axis=1) / 127.0).astype(np.float32)  # [BS]
        np.maximum(srow, 1e-30, out=srow)
        srows.append(srow)
        yn = blk / srow[:, None]                 # |yn| <= 127
        q = np.rint(yn[:, :KI])
        np.clip(q, -127, 127, out=q)
        q = q.astype(np.int8)
        # partition-major layout per chunk: tile[p, j] = q[j, ki*128 + p];
        # DVE chunks additionally biased +128 and column-interleaved so the
        # packed unpack writes halves [0:1024]=even input cols, [1024:2048].
        ytp = np.empty((128, NI8 * CW), dtype=np.int8)
        qT = q.T.reshape(NI8, 128, CW)           # [ki, p, j]
        for ci in range(NI8):
            t = qT[ci]
            if ci in dve_set:
                bt = (t.astype(np.int16) + 128).astype(np.uint8)
                it = np.empty((128, CW), dtype=np.uint8)
                it[:, 0::2] = bt[:, :CW // 2]
                it[:, 1::2] = bt[:, CW // 2:]
                ytp[:, ci * CW:(ci + 1) * CW] = it.view(np.int8)
            else:
                ytp[:, ci * CW:(ci + 1) * CW] = t
        # fp16-direct chunks 30,31: [128, 2*CW], chunk-major columns
        yftp = np.ascontiguousarray(
            yn[:, KI:].astype(np.float16).T
            .reshape(2, 128, CW).transpose(1, 0, 2).reshape(128, 2 * CW))
        in_maps.append({"y8": ytp, "yf": yftp, "w": w_ext})

    res = run_bass_kernel_spmd(_get_nc(), in_maps, list(range(NCORES)))
    _CACHE["last_result"] = res

    out = np.empty((B, MC), dtype=np.float32)
    for sh in range(NCORES):
        # ut[32*bc + c, j] = (u^T[c, bc*512 + j] / srow) / 64
        stripes = res.results[sh]["ut"].reshape(NB, 32, NFREE)[:, :MC, :]
        u = stripes.transpose(0, 2, 1).reshape(BS, MC).astype(np.float32)
        out[sh * BS:(sh + 1) * BS, :] = u * (srows[sh] * float(2 ** WSHIFT))[:, None]
    return out
